# revision 36
# baseline (speedup 1.0000x reference)
"""PointsFusion Trainium2 kernel (fp16 fast path).

Pipeline per batch b (B=4, N=4096, k=32):
  knn1 = 32-NN of p1 in p1, knn2 = 32-NN of p1 in p2 (exact, DVE 8-max rounds
  on fp32 recentred scores; dist matmul uses fp16 split-channel inputs so the
  fp32 PSUM result is exact to ~1e-6)
  gather neighbor coords, features (resi, dist) -> conv(4->64)->BN->relu
  -> conv(64->64)->BN->relu -> conv(64->128)->BN->relu -> channel-max scores
  -> softmax over 64 neighbors -> weighted sum of neighbor coords.

Sharding: 8 cores = (batch b, half h of the 4096 query points). BatchNorm uses
global batch stats -> 3 tiny AllReduces of per-channel sum/sumsq.

Layouts (per 128-query tile):
  pixel space: 16 chunks of 512; chunk c = kn*8+g, pixel o = s*16 + q
  (g = query group, q = query-in-group, s = neighbor slot, kn = which knn).
  64-ch activations y1/y2 are packed [128, 4096]: pair u = chunks (2u, 2u+1),
  chunk 2u on partitions 0:64, 2u+1 on 64:128, both at free 512u; conv1/conv2
  use block-diagonal weights so one matmul computes a whole pair.
  y3 is [128, 8192], chunk c at free 512c. Channel-max via DVE stream
  transpose (32x32 blocks) + free-axis reduce; softmax without max-subtract
  (scores bounded, exp(x-4)).

Self-contained: hardcodes shapes; no sibling imports.
"""

import sys

import numpy as np

for _p in ("/opt/trn_rl_repo", "/opt/pypackages"):
    if _p not in sys.path:
        sys.path.append(_p)

import concourse.bass as bass  # noqa: E402  (imported for side effects/typing)
import concourse.mybir as mybir  # noqa: E402
import concourse.tile as tile  # noqa: E402
from concourse import bacc, bass_isa  # noqa: E402
from concourse.bass_utils import run_bass_kernel_spmd  # noqa: E402
from concourse.masks import make_identity  # noqa: E402

F32 = mybir.dt.float32
F16 = mybir.dt.float16
U16 = mybir.dt.uint16
I16 = mybir.dt.int16
AF = mybir.ActivationFunctionType
OP = mybir.AluOpType
AX = mybir.AxisListType

NCORES = 8
B = 4
N = 4096          # candidate points per batch
KNN = 32          # neighbors per knn
QPC = 2048        # query points per core
NT = 16           # query tiles of 128 per core
C1, C2, C3 = 64, 64, 128
NTOT = float(B * N * 2 * KNN)   # BN stat count (global)
BN_EPS = 1e-3
NEG = -1.0e30
EXP_SHIFT = -4.0


def _build_program(single=False):
    nc = bacc.Bacc(
        "TRN2", target_bir_lowering=False, debug=False,
        num_devices=1 if single else NCORES,
    )
    nc._single_core_nocoll = single

    ap = {}
    def din(name, shape, dt=F16):
        ap[name] = nc.dram_tensor(name, shape, dt, kind="ExternalInput").ap()
    din("qf", [11, QPC])
    din("t1", [11, N])
    din("t2", [11, N])
    din("nqsq", [128, NT], F32)
    din("gt", [128, N], F32)
    din("gt2", [128, N], F32)
    din("qr", [4, QPC])
    din("w1b", [8, 128])
    din("w2b", [128, 128])
    din("w3d", [128, 128])
    din("gb1", [C1, 2], F32)
    din("gb2", [C2, 2], F32)
    din("gb3", [C3, 2], F32)
    din("selw", [8, 128])

    ap["out"] = nc.dram_tensor("out", [3, QPC], F32, kind="ExternalOutput").ap()

    ap["y1d"] = nc.dram_tensor("y1d", [NT, 128, 4096], F16).ap()
    ap["y2d"] = nc.dram_tensor("y2d", [NT, 128, 4096], F16).ap()
    ap["y3d"] = nc.dram_tensor("y3d", [NT, 128, 8192], F16).ap()
    ap["g1d"] = nc.dram_tensor("g1d", [NT, 128, 512], F16).ap()
    ap["g2d"] = nc.dram_tensor("g2d", [NT, 128, 512], F16).ap()
    for i, c in ((0, C1), (1, C2), (2, C3)):
        ap[f"arin{i}"] = nc.dram_tensor(f"arin{i}", [c * 2], F32).ap()
        ap[f"arout{i}"] = nc.dram_tensor(f"arout{i}", [c * 2], F32).ap()

    with tile.TileContext(nc) as tc:
        _kernel_body(tc, ap)
    nc.compile()
    return nc


def _kernel_body(tc, d):
    nc = tc.nc
    from contextlib import ExitStack

    ctx = ExitStack()
    with ctx:
        cpool = ctx.enter_context(tc.tile_pool(name="consts", bufs=1))
        w1b = cpool.tile([8, 128], F16)
        w2b = cpool.tile([128, 128], F16)
        w3d = cpool.tile([128, 128], F16)
        gb1 = cpool.tile([C1, 2], F32)
        gb2 = cpool.tile([C2, 2], F32)
        gb3 = cpool.tile([C3, 2], F32)
        selw = cpool.tile([8, 128], F16)
        for nm, sb in [("w1b", w1b), ("w2b", w2b), ("w3d", w3d),
                       ("gb1", gb1), ("gb2", gb2), ("gb3", gb3),
                       ("selw", selw)]:
            nc.sync.dma_start(out=sb[:], in_=d[nm][:])

        spool = ctx.enter_context(tc.tile_pool(name="stats", bufs=1))
        st1 = spool.tile([128, NT * 8 * 6], F32)
        st2 = spool.tile([128, NT * 8 * 6], F32)
        st3 = spool.tile([128, NT * 16 * 6], F32)
        ab1 = spool.tile([128, 2], F32)   # col0 = scale a, col1 = bias b
        ab2 = spool.tile([128, 2], F32)
        ab3 = spool.tile([C3, 2], F32)

        # ---------------- Phase 1: knn + gather + feat + conv1 ----------------
        # 3-stage software pipeline so the Vector queue never stalls:
        # iteration t issues knn(t) | feat(t-1) | conv1(t-2).
        with tc.tile_pool(name="p1c", bufs=1) as c1p, \
             tc.tile_pool(name="p1m", bufs=2) as mpool, \
             tc.tile_pool(name="p1pq", bufs=2, space="PSUM") as pq, \
             tc.tile_pool(name="p1pc", bufs=3, space="PSUM") as pc1, \
             tc.tile_pool(name="p1pt", bufs=2, space="PSUM") as pt, \
             tc.tile_pool(name="p1feat", bufs=1) as fpool, \
             tc.tile_pool(name="p1fp", bufs=2) as fppool, \
             tc.tile_pool(name="p1work", bufs=2) as wp, \
             tc.tile_pool(name="p1y", bufs=2) as yp:
            qf = c1p.tile([11, QPC], F16)
            t1 = c1p.tile([11, N], F16)
            t2 = c1p.tile([11, N], F16)
            nqsq = c1p.tile([128, NT], F32)
            gt = c1p.tile([128, N], F32)
            gt2 = c1p.tile([128, N], F32)
            qr = c1p.tile([4, QPC], F16)
            ident = c1p.tile([128, 128], F16)
            make_identity(nc, ident[:])
            for nm, sb in [("qf", qf), ("t1", t1), ("t2", t2),
                           ("nqsq", nqsq), ("gt", gt), ("gt2", gt2),
                           ("qr", qr)]:
                nc.sync.dma_start(out=sb[:], in_=d[nm][:])

            tiles = [{} for _ in range(NT)]

            def knn_stage(t):
                h = tiles[t]
                vals = wp.tile([128, 64], F32, tag="vals")
                idxu = wp.tile([128, 64], U16, tag="idxu")
                idxi = wp.tile([128, 64], I16, tag="idxi")
                h["vals"], h["idxu"], h["idxi"] = vals, idxu, idxi
                for kn, tab in ((0, t1), (1, t2)):
                    # msb = 2 q.c - |c|^2 - |q|^2 = -d^2 (max == nearest)
                    msb = mpool.tile([128, N], F32, tag="msb")
                    for ch in range(8):
                        pm = pq.tile([128, 512], F32, tag="pm")
                        nc.tensor.matmul(
                            out=pm[:],
                            lhsT=qf[:, t * 128:(t + 1) * 128],
                            rhs=tab[:, ch * 512:(ch + 1) * 512],
                            start=True, stop=True,
                        )
                        nc.scalar.activation(
                            out=msb[:, ch * 512:(ch + 1) * 512], in_=pm[:],
                            func=AF.Identity, bias=nqsq[:, t:t + 1])
                    # top-32 rounds
                    for r in range(4):
                        v8 = vals[:, kn * 32 + r * 8: kn * 32 + r * 8 + 8]
                        i8 = idxu[:, kn * 32 + r * 8: kn * 32 + r * 8 + 8]
                        nc.vector.max(out=v8, in_=msb[:])
                        nc.vector.max_index(out=i8, in_max=v8,
                                            in_values=msb[:])
                        if r < 3:
                            nc.vector.match_replace(
                                out=msb[:], in_to_replace=v8,
                                in_values=msb[:], imm_value=NEG)
                nc.vector.tensor_copy(out=idxi[:], in_=idxu[:])

            def feat_stage(t):
                h = tiles[t]
                vals, idxi = h["vals"], h["idxi"]
                # gather neighbor coords; both tables carry xyz on band rows
                # 16g+{0..2}; convert to fp16 and spill for the fusion phase
                g1 = wp.tile([128, 512], F32, tag="g1")
                g2 = wp.tile([128, 512], F32, tag="g2")
                nc.gpsimd.ap_gather(
                    out_ap=g1[:], in_ap=gt[:], idxs_ap=idxi[:, 0:32],
                    channels=128, num_elems=N, d=1, num_idxs=512)
                nc.gpsimd.ap_gather(
                    out_ap=g2[:], in_ap=gt2[:], idxs_ap=idxi[:, 32:64],
                    channels=128, num_elems=N, d=1, num_idxs=512)
                g1h = wp.tile([128, 512], F16, tag="g1h")
                g2h = wp.tile([128, 512], F16, tag="g2h")
                nc.scalar.activation(out=g1h[:], in_=g1[:], func=AF.Identity)
                nc.scalar.activation(out=g2h[:], in_=g2[:], func=AF.Identity)
                nc.sync.dma_start(out=d["g1d"][t], in_=g1h[:])
                nc.sync.dma_start(out=d["g2d"][t], in_=g2h[:])

                # feat (flat): chunk c at free 512c; band rows -> coord rows
                feat = fpool.tile([4, 8192], F16, tag="feat")
                for g in range(8):
                    nc.scalar.dma_start(
                        out=feat[0:3, g * 512:(g + 1) * 512],
                        in_=g1h[16 * g: 16 * g + 3, :])
                    nc.scalar.dma_start(
                        out=feat[0:3, (8 + g) * 512:(9 + g) * 512],
                        in_=g2h[16 * g: 16 * g + 3, :])

                # dist = sqrt(relu(-vals)) into feat row 3 (pixel layout via
                # PE transpose then per-chunk strided DMAs)
                d2h = wp.tile([128, 64], F16, tag="d2h")
                nc.scalar.activation(out=d2h[:], in_=vals[:], func=AF.Relu,
                                     scale=-1.0)
                nc.scalar.activation(out=d2h[:], in_=d2h[:], func=AF.Sqrt)
                dtp = pt.tile([64, 128], F16, tag="dtp")
                nc.tensor.transpose(out=dtp[:], in_=d2h[:], identity=ident[:])
                d2t = wp.tile([64, 128], F16, tag="d2t")
                nc.scalar.activation(out=d2t[:], in_=dtp[:], func=AF.Identity)
                for kn in (0, 1):
                    for g in range(8):
                        c = kn * 8 + g
                        nc.gpsimd.dma_start(
                            out=feat[3:4, c * 512:(c + 1) * 512]
                                .rearrange("c (s p) -> c s p", s=32),
                            in_=d2t[kn * 32:(kn + 1) * 32,
                                    16 * g:16 * g + 16])

                # resi = nn - q (in place on coord rows; gpsimd -> off the
                # DVE critical path)
                qrt = qr[0:3, t * 128:(t + 1) * 128]
                for kn in (0, 1):
                    nc.gpsimd.tensor_tensor(
                        out=feat[0:3, kn * 4096:(kn + 1) * 4096]
                            .rearrange("c (g s p) -> c g s p", g=8, s=32),
                        in0=feat[0:3, kn * 4096:(kn + 1) * 4096]
                            .rearrange("c (g s p) -> c g s p", g=8, s=32),
                        in1=qrt.rearrange("c (g p) -> c g p", g=8)
                            .unsqueeze(2).to_broadcast([3, 8, 32, 16]),
                        op=OP.subtract)

                # fold chunks into pairs: featp rows 0:4 = even chunk,
                # rows 4:8 = odd chunk, pair u at free 512u
                featp = fppool.tile([8, 4096], F16, tag="featp")
                h["featp"] = featp
                nc.gpsimd.dma_start(
                    out=featp[0:4, :].rearrange("c (u f) -> c u f", u=8),
                    in_=feat[0:4, :].rearrange("c (u f) -> c u f", u=8)
                        [:, :, 0:512])
                nc.gpsimd.dma_start(
                    out=featp[4:8, :].rearrange("c (u f) -> c u f", u=8),
                    in_=feat[0:4, :].rearrange("c (u f) -> c u f", u=8)
                        [:, :, 512:1024])

            def conv_stage(t):
                h = tiles[t]
                featp = h["featp"]
                # conv1: 8 paired matmuls -> y1 packed [128, 4096]
                y1 = yp.tile([128, 4096], F16, tag="y1")
                for u in range(8):
                    pm1 = pc1.tile([128, 512], F32, tag="pm1")
                    nc.tensor.matmul(
                        out=pm1[:], lhsT=w1b[:],
                        rhs=featp[:, u * 512:(u + 1) * 512],
                        start=True, stop=True)
                    nc.vector.bn_stats(
                        out=st1[:, (t * 8 + u) * 6:(t * 8 + u + 1) * 6],
                        in_=pm1[:])
                    nc.scalar.activation(
                        out=y1[:, u * 512:(u + 1) * 512], in_=pm1[:],
                        func=AF.Identity)
                nc.sync.dma_start(out=d["y1d"][t], in_=y1[:])
                h.clear()

            for t in range(NT):
                knn_stage(t)
                if t >= 1:
                    feat_stage(t - 1)
                if t >= 2:
                    conv_stage(t - 2)
            feat_stage(NT - 1)
            conv_stage(NT - 2)
            conv_stage(NT - 1)

        _bn_fold(tc, 0, st1, gb1, ab1, d["arin0"], d["arout0"], C1)

        # ---------------- Phase 2: apply BN1+relu, conv2 ----------------
        with tc.tile_pool(name="p2y", bufs=2) as yp, \
             tc.tile_pool(name="p2psum", bufs=6, space="PSUM") as cp:
            for t in range(NT):
                y1 = yp.tile([128, 4096], F16, tag="y1l")
                nc.sync.dma_start(out=y1[:], in_=d["y1d"][t])
                nc.scalar.activation(
                    out=y1[:], in_=y1[:], func=AF.Relu,
                    scale=ab1[:, 0:1], bias=ab1[:, 1:2])
                y2 = yp.tile([128, 4096], F16, tag="y2")
                for u in range(8):
                    pm = cp.tile([128, 512], F32, tag="pm2")
                    nc.tensor.matmul(
                        out=pm[:], lhsT=w2b[:],
                        rhs=y1[:, u * 512:(u + 1) * 512],
                        start=True, stop=True)
                    nc.vector.bn_stats(
                        out=st2[:, (t * 8 + u) * 6:(t * 8 + u + 1) * 6],
                        in_=pm[:])
                    nc.scalar.activation(
                        out=y2[:, u * 512:(u + 1) * 512], in_=pm[:],
                        func=AF.Identity)
                nc.sync.dma_start(out=d["y2d"][t], in_=y2[:])

        _bn_fold(tc, 1, st2, gb2, ab2, d["arin1"], d["arout1"], C2)

        # ---------------- Phase 3: apply BN2+relu, conv3 ----------------
        with tc.tile_pool(name="p3y", bufs=2) as yp, \
             tc.tile_pool(name="p3psum", bufs=6, space="PSUM") as cp:
            for t in range(NT):
                y2 = yp.tile([128, 4096], F16, tag="y2l")
                nc.sync.dma_start(out=y2[:], in_=d["y2d"][t])
                nc.scalar.activation(
                    out=y2[:], in_=y2[:], func=AF.Relu,
                    scale=ab2[:, 0:1], bias=ab2[:, 1:2])
                y3 = yp.tile([128, 8192], F16, tag="y3")
                for c in range(16):
                    bp_ = 64 * (c % 2)
                    pm = cp.tile([C3, 512], F32, tag="pm3")
                    nc.tensor.matmul(
                        out=pm[:], lhsT=w3d[bp_:bp_ + 64, :],
                        rhs=y2[bp_:bp_ + 64,
                               512 * (c // 2):512 * (c // 2) + 512],
                        start=True, stop=True)
                    nc.vector.bn_stats(
                        out=st3[:, (t * 16 + c) * 6:(t * 16 + c + 1) * 6],
                        in_=pm[:])
                    if c % 2 == 0:
                        nc.scalar.activation(
                            out=y3[:, c * 512:(c + 1) * 512], in_=pm[:],
                            func=AF.Identity)
                    else:
                        nc.vector.tensor_copy(
                            out=y3[:, c * 512:(c + 1) * 512], in_=pm[:])
                nc.sync.dma_start(out=d["y3d"][t], in_=y3[:])

        _bn_fold(tc, 2, st3, gb3, ab3, d["arin2"], d["arout2"], C3)

        # ------------- Phase 4: scores, softmax, fusion, output -------------
        with tc.tile_pool(name="p4z", bufs=2) as zp, \
             tc.tile_pool(name="p4zw", bufs=2) as zw, \
             tc.tile_pool(name="p4work", bufs=2) as wp, \
             tc.tile_pool(name="p4psum", bufs=2, space="PSUM") as pp4, \
             tc.tile_pool(name="p4out", bufs=1) as op_:
            outsb = op_.tile([4, QPC], F32)
            for t in range(NT):
                z = zp.tile([128, 8192], F16, tag="z")
                nc.sync.dma_start(out=z[:], in_=d["y3d"][t])
                # per-channel affine a3*z + b3 (relu deferred past the max)
                nc.scalar.activation(
                    out=z[:], in_=z[:], func=AF.Identity,
                    scale=ab3[:, 0:1], bias=ab3[:, 1:2])
                # channel max: stream-transpose 32x32 blocks first, reduce
                # each block over free (split DVE/GPSIMD), then fold the 4
                # partition groups on small [*, 256] tiles via DMA shifts
                # (engines need same-start-partition operands)
                zT = zw.tile([128, 8192], F16, tag="zT")
                nc.vector.transpose(out=zT[:], in_=z[:])
                R = wp.tile([128, 256], F16, tag="R")
                nc.vector.tensor_reduce(
                    out=R[:],
                    in_=zT[:].rearrange("c (j e) -> c j e", e=32),
                    axis=AX.X, op=OP.max)
                Rs = wp.tile([64, 256], F16, tag="Rs")
                nc.scalar.dma_start(out=Rs[:], in_=R[64:128, :])
                R2 = wp.tile([64, 256], F16, tag="R2")
                nc.vector.tensor_tensor(out=R2[:], in0=R[0:64, :],
                                        in1=Rs[:], op=OP.max)
                R2s = wp.tile([32, 256], F16, tag="R2s")
                nc.scalar.dma_start(out=R2s[:], in_=R2[32:64, :])
                T32 = wp.tile([32, 256], F16, tag="T32")
                nc.vector.tensor_tensor(out=T32[:], in0=R2[0:32, :],
                                        in1=R2s[:], op=OP.max)
                # relu (commutes with the channel max)
                nc.vector.tensor_scalar_max(T32[:], T32[:], 0.0)
                # assemble raw scores per knn half in q-major layout:
                # scX[g, q*32 + sl*16 + sh] <- T32[16*sl+q, 16*(kn*8+g)+sh]
                # (slot s = 2*sh + sl)
                scA = wp.tile([8, 512], F16, tag="scA")
                scB = wp.tile([8, 512], F16, tag="scB")
                for kn, sct in ((0, scA), (1, scB)):
                    for g in range(8):
                        cbase = 16 * (kn * 8 + g)
                        ov = sct[g:g + 1, :].rearrange(
                            "c (q sl sh) -> c sl q sh", q=16, sl=2)
                        for sl in (0, 1):
                            eng = nc.sync if (g + sl) % 2 == 0 else nc.gpsimd
                            eng.dma_start(
                                out=ov[:, sl],
                                in_=T32[16 * sl:16 * sl + 16,
                                        cbase:cbase + 16])
                # per-query max over the 64 slots, subtract, exponentiate
                qmA = wp.tile([8, 16], F16, tag="qmA")
                qmB = wp.tile([8, 16], F16, tag="qmB")
                for sct, qm in ((scA, qmA), (scB, qmB)):
                    nc.vector.tensor_reduce(
                        out=qm[:],
                        in_=sct[:].rearrange("c (q z) -> c q z", z=32),
                        axis=AX.X, op=OP.max)
                nc.vector.tensor_tensor(out=qmA[:], in0=qmA[:], in1=qmB[:],
                                        op=OP.max)
                for sct in (scA, scB):
                    nc.vector.tensor_tensor(
                        out=sct[:].rearrange("c (q z) -> c q z", z=32),
                        in0=sct[:].rearrange("c (q z) -> c q z", z=32),
                        in1=qmA[:].unsqueeze(2).to_broadcast([8, 16, 32]),
                        op=OP.subtract)
                    nc.scalar.activation(out=sct[:], in_=sct[:], func=AF.Exp)
                # denominators over the 64 slots of each query
                qsA = wp.tile([8, 16], F32, tag="qsA")
                qsB = wp.tile([8, 16], F32, tag="qsB")
                for sct, qs in ((scA, qsA), (scB, qsB)):
                    nc.vector.tensor_reduce(
                        out=qs[:],
                        in_=sct[:].rearrange("c (q z) -> c q z", z=32),
                        axis=AX.X, op=OP.add)
                nc.vector.tensor_tensor(out=qsA[:], in0=qsA[:], in1=qsB[:],
                                        op=OP.add)
                nc.vector.reciprocal(out=qsA[:], in_=qsA[:])
                rec = wp.tile([8, 16], F16, tag="rec")
                nc.vector.tensor_copy(out=rec[:], in_=qsA[:])
                for sct in (scA, scB):
                    nc.vector.tensor_tensor(
                        out=sct[:].rearrange("c (q z) -> c q z", z=32),
                        in0=sct[:].rearrange("c (q z) -> c q z", z=32),
                        in1=rec[:].unsqueeze(2).to_broadcast([8, 16, 32]),
                        op=OP.mult)
                # replicate weight rows onto band partitions, multiply with
                # raw coords, segment-reduce over slots
                wr1 = wp.tile([128, 512], F16, tag="wr1")
                wr2 = wp.tile([128, 512], F16, tag="wr2")
                for sct, wr in ((scA, wr1), (scB, wr2)):
                    pw = pp4.tile([128, 512], F32, tag="pw")
                    nc.tensor.matmul(
                        out=pw[:], lhsT=selw[:], rhs=sct[:],
                        start=True, stop=True)
                    nc.scalar.activation(out=wr[:], in_=pw[:],
                                         func=AF.Identity)
                # coords are in o = s*16+q layout -> view them q-major to
                # line up with wr (q-major from the selector matmul)
                g1l = wp.tile([128, 512], F16, tag="g1l")
                g2l = wp.tile([128, 512], F16, tag="g2l")
                nc.sync.dma_start(out=g1l[:], in_=d["g1d"][t])
                nc.sync.dma_start(out=g2l[:], in_=d["g2d"][t])
                pr = wp.tile([128, 512], F16, tag="pr")
                gv1 = g1l[:].rearrange("c (sh sl q) -> c q sl sh",
                                       sh=16, sl=2)
                gv2 = g2l[:].rearrange("c (sh sl q) -> c q sl sh",
                                       sh=16, sl=2)
                wv1 = wr1[:].rearrange("c (q sl sh) -> c q sl sh",
                                       q=16, sl=2)
                wv2 = wr2[:].rearrange("c (q sl sh) -> c q sl sh",
                                       q=16, sl=2)
                pv = pr[:].rearrange("c (q sl sh) -> c q sl sh", q=16, sl=2)
                nc.vector.tensor_tensor(out=pv, in0=gv1, in1=wv1, op=OP.mult)
                nc.gpsimd.tensor_tensor(out=wv2, in0=gv2, in1=wv2,
                                        op=OP.mult)
                nc.vector.tensor_tensor(out=pr[:], in0=pr[:], in1=wr2[:],
                                        op=OP.add)
                fp_ = wp.tile([128, 16], F32, tag="fp")
                nc.vector.tensor_reduce(
                    out=fp_[:], in_=pr[:].rearrange("c (q z) -> c q z", z=32),
                    axis=AX.X, op=OP.add)
                for g in range(8):
                    nc.scalar.dma_start(
                        out=outsb[0:3,
                                  t * 128 + 16 * g: t * 128 + 16 * g + 16],
                        in_=fp_[16 * g: 16 * g + 3, :])
            nc.sync.dma_start(out=d["out"][:], in_=outsb[0:3, :])


def _bn_fold(tc, li, st, gbe, ab, arin, arout, C):
    """bn_aggr per partition, convert to (sum, sumsq), fold dup halves for
    64-ch layers, AllReduce, then a = g*rsqrt(var+eps), b = be - a*mean."""
    nc = tc.nc
    n_loc = float(QPC * 64 * C // 128)  # pixels per partition slot
    ntot = NTOT / (NCORES if getattr(nc, "_single_core_nocoll", False) else 1)
    with tc.tile_pool(name=f"bn{li}", bufs=1) as bp:
        ag = bp.tile([128, 2], F32)
        nc.vector.bn_aggr(out=ag[:], in_=st[:])
        ss = bp.tile([128, 2], F32)
        m2 = bp.tile([128, 1], F32)
        nc.vector.tensor_tensor(out=m2[:], in0=ag[:, 0:1], in1=ag[:, 0:1],
                                op=OP.mult)
        nc.vector.tensor_tensor(out=ss[:, 1:2], in0=ag[:, 1:2], in1=m2[:],
                                op=OP.add)            # var + mean^2
        nc.vector.tensor_scalar_mul(ss[:, 1:2], ss[:, 1:2], n_loc)
        nc.vector.tensor_scalar_mul(ss[:, 0:1], ag[:, 0:1], n_loc)
        if C == 64:
            sh = bp.tile([64, 2], F32)
            nc.sync.dma_start(out=sh[:], in_=ss[64:128, :])
            sc = bp.tile([64, 2], F32)
            nc.vector.tensor_tensor(out=sc[:], in0=ss[0:64, :],
                                    in1=sh[:], op=OP.add)
        else:
            sc = ss
        nc.sync.dma_start(out=arin[:], in_=sc[:])
        if getattr(nc, "_single_core_nocoll", False):
            nc.sync.dma_start(out=arout[:], in_=arin[:])
        else:
            nc.gpsimd.collective_compute(
                "AllReduce", OP.add, replica_groups=[list(range(NCORES))],
                ins=[arin.opt()], outs=[arout.opt()])
        ar = bp.tile([C, 2], F32)
        nc.sync.dma_start(out=ar[:], in_=arout[:])
        mean = bp.tile([C, 1], F32)
        var = bp.tile([C, 1], F32)
        nc.vector.tensor_scalar_mul(mean[:], ar[:, 0:1], 1.0 / ntot)
        nc.vector.tensor_scalar_mul(var[:], ar[:, 1:2], 1.0 / ntot)
        mm = bp.tile([C, 1], F32)
        nc.vector.tensor_tensor(out=mm[:], in0=mean[:], in1=mean[:],
                                op=OP.mult)
        nc.vector.tensor_tensor(out=var[:], in0=var[:], in1=mm[:],
                                op=OP.subtract)
        nc.vector.tensor_scalar_add(var[:], var[:], BN_EPS)
        nc.scalar.activation(out=var[:], in_=var[:], func=AF.Sqrt)
        nc.vector.reciprocal(out=var[:], in_=var[:])  # rsqrt(var+eps)
        nc.vector.tensor_tensor(out=ab[0:C, 0:1], in0=var[:],
                                in1=gbe[:, 0:1], op=OP.mult)     # a
        nc.vector.tensor_tensor(out=mm[:], in0=ab[0:C, 0:1], in1=mean[:],
                                op=OP.mult)
        nc.vector.tensor_tensor(out=ab[0:C, 1:2], in0=gbe[:, 1:2], in1=mm[:],
                                op=OP.subtract)       # b = be - a*mean
        if C == 64:
            nc.vector.tensor_copy(out=ab[C:2 * C, :], in_=ab[0:C, :])


_PROGRAM = None
LAST_RESULT = None


def _get_program():
    global _PROGRAM
    if _PROGRAM is None:
        _PROGRAM = _build_program()
    return _PROGRAM


def _split16(x):
    hi = x.astype(np.float16).astype(np.float32)
    return hi, (x - hi).astype(np.float32)


def _prep_core_inputs(points1, points2, W1, W2, W3, gs, bes, b, h):
    p1 = points1[b]          # [3, N]
    p2 = points2[b]
    q = p1[:, h * QPC:(h + 1) * QPC]            # [3, QPC]

    qhi, qlo = _split16(q)
    qf = np.concatenate([2.0 * qhi, 2.0 * qhi, 2.0 * qlo,
                         np.ones((2, QPC), np.float32)], axis=0)

    def cand_tab(p):
        chi, clo = _split16(p)
        csq = (p * p).sum(axis=0)
        cshi, cslo = _split16(csq)
        return np.concatenate([chi, clo, chi, -cshi[None], -cslo[None]],
                              axis=0).astype(np.float16)   # [11, N]

    gtab = np.zeros((128, N), np.float32)
    gtab2 = np.zeros((128, N), np.float32)
    for g in range(8):
        gtab[16 * g + 0:16 * g + 3] = p1
        gtab2[16 * g + 0:16 * g + 3] = p2
    qraw = np.zeros((4, QPC), np.float16)
    qraw[0:3] = q.astype(np.float16)
    nqsqv = -(q * q).sum(axis=0).reshape(NT, 128).T.astype(np.float32)

    w1t = np.ascontiguousarray(W1.T).astype(np.float16)    # [4, 64]
    w2t = np.ascontiguousarray(W2.T).astype(np.float16)    # [64, 64]
    w3t = np.ascontiguousarray(W3.T).astype(np.float16)    # [64, 128]
    w1blk = np.zeros((8, 128), np.float16)
    w1blk[0:4, 0:64] = w1t
    w1blk[4:8, 64:128] = w1t
    w2blk = np.zeros((128, 128), np.float16)
    w2blk[0:64, 0:64] = w2t
    w2blk[64:128, 64:128] = w2t
    w3dup = np.concatenate([w3t, w3t], axis=0).astype(np.float16)

    selw = np.zeros((8, 128), np.float16)
    for g in range(8):
        for c3 in range(3):
            selw[g, 16 * g + c3] = 1.0

    return {
        "qf": qf.astype(np.float16),
        "t1": cand_tab(p1), "t2": cand_tab(p2),
        "nqsq": np.ascontiguousarray(nqsqv),
        "gt": gtab, "gt2": gtab2, "qr": qraw,
        "w1b": w1blk, "w2b": w2blk, "w3d": w3dup,
        "gb1": np.stack([gs[0], bes[0]], axis=1).astype(np.float32),
        "gb2": np.stack([gs[1], bes[1]], axis=1).astype(np.float32),
        "gb3": np.stack([gs[2], bes[2]], axis=1).astype(np.float32),
        "selw": selw,
    }


def kernel(points1, points2, k, t, W1, b1, g1, be1, W2, b2, g2, be2,
           W3, b3, g3, be3):
    # b1/b2/b3 cancel inside train-mode BatchNorm; t is unused by the net.
    assert int(np.asarray(k)) == KNN
    points1 = np.asarray(points1, np.float32)
    points2 = np.asarray(points2, np.float32)
    gs = [np.asarray(g1, np.float32), np.asarray(g2, np.float32),
          np.asarray(g3, np.float32)]
    bes = [np.asarray(be1, np.float32), np.asarray(be2, np.float32),
           np.asarray(be3, np.float32)]
    Ws = [np.asarray(W1, np.float32), np.asarray(W2, np.float32),
          np.asarray(W3, np.float32)]

    in_maps = []
    for c in range(NCORES):
        b, h = divmod(c, 2)
        in_maps.append(_prep_core_inputs(points1, points2, *Ws, gs, bes, b, h))

    nc = _get_program()
    bkr = run_bass_kernel_spmd(nc, in_maps, list(range(NCORES)))
    global LAST_RESULT
    LAST_RESULT = bkr
    res = bkr.results

    out = np.zeros((B, 3, N), np.float32)
    for c in range(NCORES):
        b, h = divmod(c, 2)
        out[b, :, h * QPC:(h + 1) * QPC] = res[c]["out"]
    return out


# revision 38
# speedup vs baseline: 1.0305x; 1.0305x over previous
"""PointsFusion Trainium2 kernel (fp16 fast path).

Pipeline per batch b (B=4, N=4096, k=32):
  knn1 = 32-NN of p1 in p1, knn2 = 32-NN of p1 in p2 (exact, DVE 8-max rounds
  on fp32 recentred scores; dist matmul uses fp16 split-channel inputs so the
  fp32 PSUM result is exact to ~1e-6)
  gather neighbor coords, features (resi, dist) -> conv(4->64)->BN->relu
  -> conv(64->64)->BN->relu -> conv(64->128)->BN->relu -> channel-max scores
  -> softmax over 64 neighbors -> weighted sum of neighbor coords.

Sharding: 8 cores = (batch b, half h of the 4096 query points). BatchNorm uses
global batch stats -> 3 tiny AllReduces of per-channel sum/sumsq.

Layouts (per 128-query tile):
  pixel space: 16 chunks of 512; chunk c = kn*8+g, pixel o = s*16 + q
  (g = query group, q = query-in-group, s = neighbor slot, kn = which knn).
  64-ch activations y1/y2 are packed [128, 4096]: pair u = chunks (2u, 2u+1),
  chunk 2u on partitions 0:64, 2u+1 on 64:128, both at free 512u; conv1/conv2
  use block-diagonal weights so one matmul computes a whole pair.
  y3 is [128, 8192], chunk c at free 512c. Channel-max via DVE stream
  transpose (32x32 blocks) + free-axis reduce; softmax without max-subtract
  (scores bounded, exp(x-4)).

Self-contained: hardcodes shapes; no sibling imports.
"""

import sys

import numpy as np

for _p in ("/opt/trn_rl_repo", "/opt/pypackages"):
    if _p not in sys.path:
        sys.path.append(_p)

import concourse.bass as bass  # noqa: E402  (imported for side effects/typing)
import concourse.mybir as mybir  # noqa: E402
import concourse.tile as tile  # noqa: E402
from concourse import bacc, bass_isa  # noqa: E402
from concourse.bass_utils import run_bass_kernel_spmd  # noqa: E402
from concourse.masks import make_identity  # noqa: E402

F32 = mybir.dt.float32
F16 = mybir.dt.float16
U16 = mybir.dt.uint16
I16 = mybir.dt.int16
AF = mybir.ActivationFunctionType
OP = mybir.AluOpType
AX = mybir.AxisListType

NCORES = 8
B = 4
N = 4096          # candidate points per batch
KNN = 32          # neighbors per knn
QPC = 2048        # query points per core
NT = 16           # query tiles of 128 per core
C1, C2, C3 = 64, 64, 128
NTOT = float(B * N * 2 * KNN)   # BN stat count (global)
BN_EPS = 1e-3
NEG = -1.0e30
EXP_SHIFT = -4.0


def _build_program(single=False):
    nc = bacc.Bacc(
        "TRN2", target_bir_lowering=False, debug=False,
        num_devices=1 if single else NCORES,
    )
    nc._single_core_nocoll = single

    ap = {}
    def din(name, shape, dt=F16):
        ap[name] = nc.dram_tensor(name, shape, dt, kind="ExternalInput").ap()
    din("qf", [11, QPC])
    din("t1", [11, N])
    din("t2", [11, N])
    din("nqsq", [128, NT], F32)
    din("gt", [128, N], F32)
    din("gt2", [128, N], F32)
    din("qr", [4, QPC])
    din("w1b", [8, 128])
    din("w2b", [128, 128])
    din("w3d", [128, 128])
    din("gb1", [C1, 2], F32)
    din("gb2", [C2, 2], F32)
    din("gb3", [C3, 2], F32)
    din("selw", [8, 128])

    ap["out"] = nc.dram_tensor("out", [3, QPC], F32, kind="ExternalOutput").ap()

    ap["y1d"] = nc.dram_tensor("y1d", [NT, 128, 4096], F16).ap()
    ap["y2d"] = nc.dram_tensor("y2d", [NT, 128, 4096], F16).ap()
    ap["y3d"] = nc.dram_tensor("y3d", [NT, 128, 8192], F16).ap()
    ap["g1d"] = nc.dram_tensor("g1d", [NT, 128, 512], F16).ap()
    ap["g2d"] = nc.dram_tensor("g2d", [NT, 128, 512], F16).ap()
    for i, c in ((0, C1), (1, C2), (2, C3)):
        ap[f"arin{i}"] = nc.dram_tensor(f"arin{i}", [c * 2], F32).ap()
        ap[f"arout{i}"] = nc.dram_tensor(f"arout{i}", [c * 2], F32).ap()

    with tile.TileContext(nc) as tc:
        _kernel_body(tc, ap)
    nc.compile()
    return nc


def _kernel_body(tc, d):
    nc = tc.nc
    from contextlib import ExitStack

    ctx = ExitStack()
    with ctx:
        cpool = ctx.enter_context(tc.tile_pool(name="consts", bufs=1))
        w1b = cpool.tile([8, 128], F16)
        w2b = cpool.tile([128, 128], F16)
        w3d = cpool.tile([128, 128], F16)
        gb1 = cpool.tile([C1, 2], F32)
        gb2 = cpool.tile([C2, 2], F32)
        gb3 = cpool.tile([C3, 2], F32)
        selw = cpool.tile([8, 128], F16)
        for nm, sb in [("w1b", w1b), ("w2b", w2b), ("w3d", w3d),
                       ("gb1", gb1), ("gb2", gb2), ("gb3", gb3),
                       ("selw", selw)]:
            nc.sync.dma_start(out=sb[:], in_=d[nm][:])

        spool = ctx.enter_context(tc.tile_pool(name="stats", bufs=1))
        st1 = spool.tile([128, NT * 8 * 6], F32)
        st2 = spool.tile([128, NT * 8 * 6], F32)
        st3 = spool.tile([128, NT * 16 * 6], F32)
        ab1 = spool.tile([128, 2], F32)   # col0 = scale a, col1 = bias b
        ab2 = spool.tile([128, 2], F32)
        ab3 = spool.tile([C3, 2], F32)

        # ---------------- Phase 1: knn + gather + feat + conv1 ----------------
        # 3-stage software pipeline so the Vector queue never stalls:
        # iteration t issues knn(t) | feat(t-1) | conv1(t-2).
        with tc.tile_pool(name="p1c", bufs=1) as c1p, \
             tc.tile_pool(name="p1m", bufs=2) as mpool, \
             tc.tile_pool(name="p1pq", bufs=2, space="PSUM") as pq, \
             tc.tile_pool(name="p1pc", bufs=3, space="PSUM") as pc1, \
             tc.tile_pool(name="p1pt", bufs=2, space="PSUM") as pt, \
             tc.tile_pool(name="p1feat", bufs=1) as fpool, \
             tc.tile_pool(name="p1fp", bufs=2) as fppool, \
             tc.tile_pool(name="p1work", bufs=2) as wp, \
             tc.tile_pool(name="p1y", bufs=2) as yp:
            qf = c1p.tile([11, QPC], F16)
            t1 = c1p.tile([11, N], F16)
            t2 = c1p.tile([11, N], F16)
            nqsq = c1p.tile([128, NT], F32)
            gt = c1p.tile([128, N], F32)
            gt2 = c1p.tile([128, N], F32)
            qr = c1p.tile([4, QPC], F16)
            ident = c1p.tile([128, 128], F16)
            make_identity(nc, ident[:])
            for nm, sb in [("qf", qf), ("t1", t1), ("t2", t2),
                           ("nqsq", nqsq), ("gt", gt), ("gt2", gt2),
                           ("qr", qr)]:
                nc.sync.dma_start(out=sb[:], in_=d[nm][:])

            tiles = [{} for _ in range(NT)]

            def knn_stage(t):
                h = tiles[t]
                vals = wp.tile([128, 64], F32, tag="vals")
                idxu = wp.tile([128, 64], U16, tag="idxu")
                idxi = wp.tile([128, 64], I16, tag="idxi")
                h["vals"], h["idxu"], h["idxi"] = vals, idxu, idxi
                for kn, tab in ((0, t1), (1, t2)):
                    # msb = 2 q.c - |c|^2 - |q|^2 = -d^2 (max == nearest)
                    msb = mpool.tile([128, N], F32, tag="msb")
                    for ch in range(8):
                        pm = pq.tile([128, 512], F32, tag="pm")
                        nc.tensor.matmul(
                            out=pm[:],
                            lhsT=qf[:, t * 128:(t + 1) * 128],
                            rhs=tab[:, ch * 512:(ch + 1) * 512],
                            start=True, stop=True,
                        )
                        nc.scalar.activation(
                            out=msb[:, ch * 512:(ch + 1) * 512], in_=pm[:],
                            func=AF.Identity, bias=nqsq[:, t:t + 1])
                    # top-32 rounds
                    for r in range(4):
                        v8 = vals[:, kn * 32 + r * 8: kn * 32 + r * 8 + 8]
                        i8 = idxu[:, kn * 32 + r * 8: kn * 32 + r * 8 + 8]
                        nc.vector.max(out=v8, in_=msb[:])
                        nc.vector.max_index(out=i8, in_max=v8,
                                            in_values=msb[:])
                        if r < 3:
                            nc.vector.match_replace(
                                out=msb[:], in_to_replace=v8,
                                in_values=msb[:], imm_value=NEG)
                nc.vector.tensor_copy(out=idxi[:], in_=idxu[:])

            def feat_stage(t):
                h = tiles[t]
                vals, idxi = h["vals"], h["idxi"]
                # gather neighbor coords; both tables carry xyz on band rows
                # 16g+{0..2}; convert to fp16 and spill for the fusion phase
                g1 = wp.tile([128, 512], F32, tag="g1")
                g2 = wp.tile([128, 512], F32, tag="g2")
                nc.gpsimd.ap_gather(
                    out_ap=g1[:], in_ap=gt[:], idxs_ap=idxi[:, 0:32],
                    channels=128, num_elems=N, d=1, num_idxs=512)
                nc.gpsimd.ap_gather(
                    out_ap=g2[:], in_ap=gt2[:], idxs_ap=idxi[:, 32:64],
                    channels=128, num_elems=N, d=1, num_idxs=512)
                g1h = wp.tile([128, 512], F16, tag="g1h")
                g2h = wp.tile([128, 512], F16, tag="g2h")
                nc.scalar.activation(out=g1h[:], in_=g1[:], func=AF.Identity)
                nc.scalar.activation(out=g2h[:], in_=g2[:], func=AF.Identity)
                nc.sync.dma_start(out=d["g1d"][t], in_=g1h[:])
                nc.sync.dma_start(out=d["g2d"][t], in_=g2h[:])

                # feat (flat): chunk c at free 512c; band rows -> coord rows
                feat = fpool.tile([4, 8192], F16, tag="feat")
                for g in range(8):
                    nc.scalar.dma_start(
                        out=feat[0:3, g * 512:(g + 1) * 512],
                        in_=g1h[16 * g: 16 * g + 3, :])
                    nc.scalar.dma_start(
                        out=feat[0:3, (8 + g) * 512:(9 + g) * 512],
                        in_=g2h[16 * g: 16 * g + 3, :])

                # dist = sqrt(relu(-vals)) into feat row 3 (pixel layout via
                # PE transpose then per-chunk strided DMAs)
                d2h = wp.tile([128, 64], F16, tag="d2h")
                nc.scalar.activation(out=d2h[:], in_=vals[:], func=AF.Relu,
                                     scale=-1.0)
                nc.scalar.activation(out=d2h[:], in_=d2h[:], func=AF.Sqrt)
                dtp = pt.tile([64, 128], F16, tag="dtp")
                nc.tensor.transpose(out=dtp[:], in_=d2h[:], identity=ident[:])
                d2t = wp.tile([64, 128], F16, tag="d2t")
                nc.scalar.activation(out=d2t[:], in_=dtp[:], func=AF.Identity)
                for kn in (0, 1):
                    for g in range(8):
                        c = kn * 8 + g
                        nc.gpsimd.dma_start(
                            out=feat[3:4, c * 512:(c + 1) * 512]
                                .rearrange("c (s p) -> c s p", s=32),
                            in_=d2t[kn * 32:(kn + 1) * 32,
                                    16 * g:16 * g + 16])

                # resi = nn - q (in place on coord rows; gpsimd -> off the
                # DVE critical path)
                qrt = qr[0:3, t * 128:(t + 1) * 128]
                for kn in (0, 1):
                    nc.gpsimd.tensor_tensor(
                        out=feat[0:3, kn * 4096:(kn + 1) * 4096]
                            .rearrange("c (g s p) -> c g s p", g=8, s=32),
                        in0=feat[0:3, kn * 4096:(kn + 1) * 4096]
                            .rearrange("c (g s p) -> c g s p", g=8, s=32),
                        in1=qrt.rearrange("c (g p) -> c g p", g=8)
                            .unsqueeze(2).to_broadcast([3, 8, 32, 16]),
                        op=OP.subtract)

                # fold chunks into pairs: featp rows 0:4 = even chunk,
                # rows 4:8 = odd chunk, pair u at free 512u
                featp = fppool.tile([8, 4096], F16, tag="featp")
                h["featp"] = featp
                nc.gpsimd.dma_start(
                    out=featp[0:4, :].rearrange("c (u f) -> c u f", u=8),
                    in_=feat[0:4, :].rearrange("c (u f) -> c u f", u=8)
                        [:, :, 0:512])
                nc.gpsimd.dma_start(
                    out=featp[4:8, :].rearrange("c (u f) -> c u f", u=8),
                    in_=feat[0:4, :].rearrange("c (u f) -> c u f", u=8)
                        [:, :, 512:1024])

            def conv_stage(t):
                h = tiles[t]
                featp = h["featp"]
                # conv1: 8 paired matmuls -> y1 packed [128, 4096]
                y1 = yp.tile([128, 4096], F16, tag="y1")
                for u in range(8):
                    pm1 = pc1.tile([128, 512], F32, tag="pm1")
                    nc.tensor.matmul(
                        out=pm1[:], lhsT=w1b[:],
                        rhs=featp[:, u * 512:(u + 1) * 512],
                        start=True, stop=True)
                    nc.vector.bn_stats(
                        out=st1[:, (t * 8 + u) * 6:(t * 8 + u + 1) * 6],
                        in_=pm1[:])
                    nc.scalar.activation(
                        out=y1[:, u * 512:(u + 1) * 512], in_=pm1[:],
                        func=AF.Identity)
                nc.sync.dma_start(out=d["y1d"][t], in_=y1[:])
                h.clear()

            for t in range(NT):
                knn_stage(t)
                if t >= 1:
                    feat_stage(t - 1)
                if t >= 2:
                    conv_stage(t - 2)
            feat_stage(NT - 1)
            conv_stage(NT - 2)
            conv_stage(NT - 1)

        _bn_fold(tc, 0, st1, gb1, ab1, d["arin0"], d["arout0"], C1)

        # ---------------- Phase 2: apply BN1+relu, conv2 ----------------
        with tc.tile_pool(name="p2y", bufs=2) as yp, \
             tc.tile_pool(name="p2psum", bufs=6, space="PSUM") as cp:
            for t in range(NT):
                y1 = yp.tile([128, 4096], F16, tag="y1l")
                nc.sync.dma_start(out=y1[:], in_=d["y1d"][t])
                nc.scalar.activation(
                    out=y1[:], in_=y1[:], func=AF.Relu,
                    scale=ab1[:, 0:1], bias=ab1[:, 1:2])
                y2 = yp.tile([128, 4096], F16, tag="y2")
                for u in range(8):
                    pm = cp.tile([128, 512], F32, tag="pm2")
                    nc.tensor.matmul(
                        out=pm[:], lhsT=w2b[:],
                        rhs=y1[:, u * 512:(u + 1) * 512],
                        start=True, stop=True)
                    nc.vector.bn_stats(
                        out=st2[:, (t * 8 + u) * 6:(t * 8 + u + 1) * 6],
                        in_=pm[:])
                    # 5 copies on scalar, 3 on vector: P2 is scalar-bound
                    if u % 3 != 2:
                        nc.scalar.activation(
                            out=y2[:, u * 512:(u + 1) * 512], in_=pm[:],
                            func=AF.Identity)
                    else:
                        nc.vector.tensor_copy(
                            out=y2[:, u * 512:(u + 1) * 512], in_=pm[:])
                nc.sync.dma_start(out=d["y2d"][t], in_=y2[:])

        _bn_fold(tc, 1, st2, gb2, ab2, d["arin1"], d["arout1"], C2)

        # ---------------- Phase 3: apply BN2+relu, conv3 ----------------
        with tc.tile_pool(name="p3y", bufs=2) as yp, \
             tc.tile_pool(name="p3psum", bufs=6, space="PSUM") as cp:
            for t in range(NT):
                y2 = yp.tile([128, 4096], F16, tag="y2l")
                nc.sync.dma_start(out=y2[:], in_=d["y2d"][t])
                nc.scalar.activation(
                    out=y2[:], in_=y2[:], func=AF.Relu,
                    scale=ab2[:, 0:1], bias=ab2[:, 1:2])
                y3 = yp.tile([128, 8192], F16, tag="y3")
                for c in range(16):
                    bp_ = 64 * (c % 2)
                    pm = cp.tile([C3, 512], F32, tag="pm3")
                    nc.tensor.matmul(
                        out=pm[:], lhsT=w3d[bp_:bp_ + 64, :],
                        rhs=y2[bp_:bp_ + 64,
                               512 * (c // 2):512 * (c // 2) + 512],
                        start=True, stop=True)
                    nc.vector.bn_stats(
                        out=st3[:, (t * 16 + c) * 6:(t * 16 + c + 1) * 6],
                        in_=pm[:])
                    # 12 copies on scalar, 4 on vector: balances S vs the
                    # bn_stats-loaded DVE
                    if c % 4 != 3:
                        nc.scalar.activation(
                            out=y3[:, c * 512:(c + 1) * 512], in_=pm[:],
                            func=AF.Identity)
                    else:
                        nc.vector.tensor_copy(
                            out=y3[:, c * 512:(c + 1) * 512], in_=pm[:])
                nc.sync.dma_start(out=d["y3d"][t], in_=y3[:])

        _bn_fold(tc, 2, st3, gb3, ab3, d["arin2"], d["arout2"], C3)

        # ------------- Phase 4: scores, softmax, fusion, output -------------
        with tc.tile_pool(name="p4z", bufs=2) as zp, \
             tc.tile_pool(name="p4zw", bufs=2) as zw, \
             tc.tile_pool(name="p4work", bufs=2) as wp, \
             tc.tile_pool(name="p4psum", bufs=2, space="PSUM") as pp4, \
             tc.tile_pool(name="p4out", bufs=1) as op_:
            outsb = op_.tile([4, QPC], F32)
            for t in range(NT):
                z = zp.tile([128, 8192], F16, tag="z")
                nc.sync.dma_start(out=z[:], in_=d["y3d"][t])
                # per-channel affine a3*z + b3 (relu deferred past the max)
                nc.scalar.activation(
                    out=z[:], in_=z[:], func=AF.Identity,
                    scale=ab3[:, 0:1], bias=ab3[:, 1:2])
                # channel max: stream-transpose 32x32 blocks first, reduce
                # each block over free (split DVE/GPSIMD), then fold the 4
                # partition groups on small [*, 256] tiles via DMA shifts
                # (engines need same-start-partition operands)
                zT = zw.tile([128, 8192], F16, tag="zT")
                nc.vector.transpose(out=zT[:], in_=z[:])
                R = wp.tile([128, 256], F16, tag="R")
                nc.vector.tensor_reduce(
                    out=R[:],
                    in_=zT[:].rearrange("c (j e) -> c j e", e=32),
                    axis=AX.X, op=OP.max)
                Rs = wp.tile([64, 256], F16, tag="Rs")
                nc.scalar.dma_start(out=Rs[:], in_=R[64:128, :])
                R2 = wp.tile([64, 256], F16, tag="R2")
                nc.vector.tensor_tensor(out=R2[:], in0=R[0:64, :],
                                        in1=Rs[:], op=OP.max)
                R2s = wp.tile([32, 256], F16, tag="R2s")
                nc.scalar.dma_start(out=R2s[:], in_=R2[32:64, :])
                T32 = wp.tile([32, 256], F16, tag="T32")
                nc.vector.tensor_tensor(out=T32[:], in0=R2[0:32, :],
                                        in1=R2s[:], op=OP.max)
                # relu (commutes with the channel max)
                nc.vector.tensor_scalar_max(T32[:], T32[:], 0.0)
                # assemble raw scores per knn half in q-major layout:
                # scX[g, q*32 + sl*16 + sh] <- T32[16*sl+q, 16*(kn*8+g)+sh]
                # (slot s = 2*sh + sl)
                scA = wp.tile([8, 512], F16, tag="scA")
                scB = wp.tile([8, 512], F16, tag="scB")
                for kn, sct in ((0, scA), (1, scB)):
                    for g in range(8):
                        cbase = 16 * (kn * 8 + g)
                        ov = sct[g:g + 1, :].rearrange(
                            "c (q sl sh) -> c sl q sh", q=16, sl=2)
                        for sl in (0, 1):
                            eng = nc.sync if (g + sl) % 2 == 0 else nc.gpsimd
                            eng.dma_start(
                                out=ov[:, sl],
                                in_=T32[16 * sl:16 * sl + 16,
                                        cbase:cbase + 16])
                # per-query max over the 64 slots, subtract, exponentiate
                qmA = wp.tile([8, 16], F16, tag="qmA")
                qmB = wp.tile([8, 16], F16, tag="qmB")
                for sct, qm in ((scA, qmA), (scB, qmB)):
                    nc.vector.tensor_reduce(
                        out=qm[:],
                        in_=sct[:].rearrange("c (q z) -> c q z", z=32),
                        axis=AX.X, op=OP.max)
                nc.vector.tensor_tensor(out=qmA[:], in0=qmA[:], in1=qmB[:],
                                        op=OP.max)
                for sct in (scA, scB):
                    nc.vector.tensor_tensor(
                        out=sct[:].rearrange("c (q z) -> c q z", z=32),
                        in0=sct[:].rearrange("c (q z) -> c q z", z=32),
                        in1=qmA[:].unsqueeze(2).to_broadcast([8, 16, 32]),
                        op=OP.subtract)
                    nc.scalar.activation(out=sct[:], in_=sct[:], func=AF.Exp)
                # denominators over the 64 slots of each query
                qsA = wp.tile([8, 16], F32, tag="qsA")
                qsB = wp.tile([8, 16], F32, tag="qsB")
                for sct, qs in ((scA, qsA), (scB, qsB)):
                    nc.vector.tensor_reduce(
                        out=qs[:],
                        in_=sct[:].rearrange("c (q z) -> c q z", z=32),
                        axis=AX.X, op=OP.add)
                nc.vector.tensor_tensor(out=qsA[:], in0=qsA[:], in1=qsB[:],
                                        op=OP.add)
                nc.vector.reciprocal(out=qsA[:], in_=qsA[:])
                rec = wp.tile([8, 16], F16, tag="rec")
                nc.vector.tensor_copy(out=rec[:], in_=qsA[:])
                for sct in (scA, scB):
                    nc.vector.tensor_tensor(
                        out=sct[:].rearrange("c (q z) -> c q z", z=32),
                        in0=sct[:].rearrange("c (q z) -> c q z", z=32),
                        in1=rec[:].unsqueeze(2).to_broadcast([8, 16, 32]),
                        op=OP.mult)
                # replicate weight rows onto band partitions, multiply with
                # raw coords, segment-reduce over slots
                wr1 = wp.tile([128, 512], F16, tag="wr1")
                wr2 = wp.tile([128, 512], F16, tag="wr2")
                for sct, wr in ((scA, wr1), (scB, wr2)):
                    pw = pp4.tile([128, 512], F32, tag="pw")
                    nc.tensor.matmul(
                        out=pw[:], lhsT=selw[:], rhs=sct[:],
                        start=True, stop=True)
                    nc.scalar.activation(out=wr[:], in_=pw[:],
                                         func=AF.Identity)
                # coords are in o = s*16+q layout -> view them q-major to
                # line up with wr (q-major from the selector matmul)
                g1l = wp.tile([128, 512], F16, tag="g1l")
                g2l = wp.tile([128, 512], F16, tag="g2l")
                nc.sync.dma_start(out=g1l[:], in_=d["g1d"][t])
                nc.sync.dma_start(out=g2l[:], in_=d["g2d"][t])
                pr = wp.tile([128, 512], F16, tag="pr")
                gv1 = g1l[:].rearrange("c (sh sl q) -> c q sl sh",
                                       sh=16, sl=2)
                gv2 = g2l[:].rearrange("c (sh sl q) -> c q sl sh",
                                       sh=16, sl=2)
                wv1 = wr1[:].rearrange("c (q sl sh) -> c q sl sh",
                                       q=16, sl=2)
                wv2 = wr2[:].rearrange("c (q sl sh) -> c q sl sh",
                                       q=16, sl=2)
                pv = pr[:].rearrange("c (q sl sh) -> c q sl sh", q=16, sl=2)
                nc.vector.tensor_tensor(out=pv, in0=gv1, in1=wv1, op=OP.mult)
                nc.gpsimd.tensor_tensor(out=wv2, in0=gv2, in1=wv2,
                                        op=OP.mult)
                nc.vector.tensor_tensor(out=pr[:], in0=pr[:], in1=wr2[:],
                                        op=OP.add)
                fp_ = wp.tile([128, 16], F32, tag="fp")
                nc.vector.tensor_reduce(
                    out=fp_[:], in_=pr[:].rearrange("c (q z) -> c q z", z=32),
                    axis=AX.X, op=OP.add)
                for g in range(8):
                    nc.scalar.dma_start(
                        out=outsb[0:3,
                                  t * 128 + 16 * g: t * 128 + 16 * g + 16],
                        in_=fp_[16 * g: 16 * g + 3, :])
            nc.sync.dma_start(out=d["out"][:], in_=outsb[0:3, :])


def _bn_fold(tc, li, st, gbe, ab, arin, arout, C):
    """bn_aggr per partition, convert to (sum, sumsq), fold dup halves for
    64-ch layers, AllReduce, then a = g*rsqrt(var+eps), b = be - a*mean."""
    nc = tc.nc
    n_loc = float(QPC * 64 * C // 128)  # pixels per partition slot
    ntot = NTOT / (NCORES if getattr(nc, "_single_core_nocoll", False) else 1)
    with tc.tile_pool(name=f"bn{li}", bufs=1) as bp:
        ag = bp.tile([128, 2], F32)
        nc.vector.bn_aggr(out=ag[:], in_=st[:])
        ss = bp.tile([128, 2], F32)
        m2 = bp.tile([128, 1], F32)
        nc.vector.tensor_tensor(out=m2[:], in0=ag[:, 0:1], in1=ag[:, 0:1],
                                op=OP.mult)
        nc.vector.tensor_tensor(out=ss[:, 1:2], in0=ag[:, 1:2], in1=m2[:],
                                op=OP.add)            # var + mean^2
        nc.vector.tensor_scalar_mul(ss[:, 1:2], ss[:, 1:2], n_loc)
        nc.vector.tensor_scalar_mul(ss[:, 0:1], ag[:, 0:1], n_loc)
        if C == 64:
            sh = bp.tile([64, 2], F32)
            nc.sync.dma_start(out=sh[:], in_=ss[64:128, :])
            sc = bp.tile([64, 2], F32)
            nc.vector.tensor_tensor(out=sc[:], in0=ss[0:64, :],
                                    in1=sh[:], op=OP.add)
        else:
            sc = ss
        nc.sync.dma_start(out=arin[:], in_=sc[:])
        if getattr(nc, "_single_core_nocoll", False):
            nc.sync.dma_start(out=arout[:], in_=arin[:])
        else:
            nc.gpsimd.collective_compute(
                "AllReduce", OP.add, replica_groups=[list(range(NCORES))],
                ins=[arin.opt()], outs=[arout.opt()])
        ar = bp.tile([C, 2], F32)
        nc.sync.dma_start(out=ar[:], in_=arout[:])
        mean = bp.tile([C, 1], F32)
        var = bp.tile([C, 1], F32)
        nc.vector.tensor_scalar_mul(mean[:], ar[:, 0:1], 1.0 / ntot)
        nc.vector.tensor_scalar_mul(var[:], ar[:, 1:2], 1.0 / ntot)
        mm = bp.tile([C, 1], F32)
        nc.vector.tensor_tensor(out=mm[:], in0=mean[:], in1=mean[:],
                                op=OP.mult)
        nc.vector.tensor_tensor(out=var[:], in0=var[:], in1=mm[:],
                                op=OP.subtract)
        nc.vector.tensor_scalar_add(var[:], var[:], BN_EPS)
        nc.scalar.activation(out=var[:], in_=var[:], func=AF.Sqrt)
        nc.vector.reciprocal(out=var[:], in_=var[:])  # rsqrt(var+eps)
        nc.vector.tensor_tensor(out=ab[0:C, 0:1], in0=var[:],
                                in1=gbe[:, 0:1], op=OP.mult)     # a
        nc.vector.tensor_tensor(out=mm[:], in0=ab[0:C, 0:1], in1=mean[:],
                                op=OP.mult)
        nc.vector.tensor_tensor(out=ab[0:C, 1:2], in0=gbe[:, 1:2], in1=mm[:],
                                op=OP.subtract)       # b = be - a*mean
        if C == 64:
            nc.vector.tensor_copy(out=ab[C:2 * C, :], in_=ab[0:C, :])


_PROGRAM = None
LAST_RESULT = None


def _get_program():
    global _PROGRAM
    if _PROGRAM is None:
        _PROGRAM = _build_program()
    return _PROGRAM


def _split16(x):
    hi = x.astype(np.float16).astype(np.float32)
    return hi, (x - hi).astype(np.float32)


def _prep_core_inputs(points1, points2, W1, W2, W3, gs, bes, b, h):
    p1 = points1[b]          # [3, N]
    p2 = points2[b]
    q = p1[:, h * QPC:(h + 1) * QPC]            # [3, QPC]

    qhi, qlo = _split16(q)
    qf = np.concatenate([2.0 * qhi, 2.0 * qhi, 2.0 * qlo,
                         np.ones((2, QPC), np.float32)], axis=0)

    def cand_tab(p):
        chi, clo = _split16(p)
        csq = (p * p).sum(axis=0)
        cshi, cslo = _split16(csq)
        return np.concatenate([chi, clo, chi, -cshi[None], -cslo[None]],
                              axis=0).astype(np.float16)   # [11, N]

    gtab = np.zeros((128, N), np.float32)
    gtab2 = np.zeros((128, N), np.float32)
    for g in range(8):
        gtab[16 * g + 0:16 * g + 3] = p1
        gtab2[16 * g + 0:16 * g + 3] = p2
    qraw = np.zeros((4, QPC), np.float16)
    qraw[0:3] = q.astype(np.float16)
    nqsqv = -(q * q).sum(axis=0).reshape(NT, 128).T.astype(np.float32)

    w1t = np.ascontiguousarray(W1.T).astype(np.float16)    # [4, 64]
    w2t = np.ascontiguousarray(W2.T).astype(np.float16)    # [64, 64]
    w3t = np.ascontiguousarray(W3.T).astype(np.float16)    # [64, 128]
    w1blk = np.zeros((8, 128), np.float16)
    w1blk[0:4, 0:64] = w1t
    w1blk[4:8, 64:128] = w1t
    w2blk = np.zeros((128, 128), np.float16)
    w2blk[0:64, 0:64] = w2t
    w2blk[64:128, 64:128] = w2t
    w3dup = np.concatenate([w3t, w3t], axis=0).astype(np.float16)

    selw = np.zeros((8, 128), np.float16)
    for g in range(8):
        for c3 in range(3):
            selw[g, 16 * g + c3] = 1.0

    return {
        "qf": qf.astype(np.float16),
        "t1": cand_tab(p1), "t2": cand_tab(p2),
        "nqsq": np.ascontiguousarray(nqsqv),
        "gt": gtab, "gt2": gtab2, "qr": qraw,
        "w1b": w1blk, "w2b": w2blk, "w3d": w3dup,
        "gb1": np.stack([gs[0], bes[0]], axis=1).astype(np.float32),
        "gb2": np.stack([gs[1], bes[1]], axis=1).astype(np.float32),
        "gb3": np.stack([gs[2], bes[2]], axis=1).astype(np.float32),
        "selw": selw,
    }


def kernel(points1, points2, k, t, W1, b1, g1, be1, W2, b2, g2, be2,
           W3, b3, g3, be3):
    # b1/b2/b3 cancel inside train-mode BatchNorm; t is unused by the net.
    assert int(np.asarray(k)) == KNN
    points1 = np.asarray(points1, np.float32)
    points2 = np.asarray(points2, np.float32)
    gs = [np.asarray(g1, np.float32), np.asarray(g2, np.float32),
          np.asarray(g3, np.float32)]
    bes = [np.asarray(be1, np.float32), np.asarray(be2, np.float32),
           np.asarray(be3, np.float32)]
    Ws = [np.asarray(W1, np.float32), np.asarray(W2, np.float32),
          np.asarray(W3, np.float32)]

    in_maps = []
    for c in range(NCORES):
        b, h = divmod(c, 2)
        in_maps.append(_prep_core_inputs(points1, points2, *Ws, gs, bes, b, h))

    nc = _get_program()
    bkr = run_bass_kernel_spmd(nc, in_maps, list(range(NCORES)))
    global LAST_RESULT
    LAST_RESULT = bkr
    res = bkr.results

    out = np.zeros((B, 3, N), np.float32)
    for c in range(NCORES):
        b, h = divmod(c, 2)
        out[b, :, h * QPC:(h + 1) * QPC] = res[c]["out"]
    return out


# revision 42
# speedup vs baseline: 1.0656x; 1.0341x over previous
"""PointsFusion Trainium2 kernel (fp16 fast path).

Pipeline per batch b (B=4, N=4096, k=32):
  knn1 = 32-NN of p1 in p1, knn2 = 32-NN of p1 in p2 (exact, DVE 8-max rounds
  on fp32 recentred scores; dist matmul uses fp16 split-channel inputs so the
  fp32 PSUM result is exact to ~1e-6)
  gather neighbor coords, features (resi, dist) -> conv(4->64)->BN->relu
  -> conv(64->64)->BN->relu -> conv(64->128)->BN->relu -> channel-max scores
  -> softmax over 64 neighbors -> weighted sum of neighbor coords.

Sharding: 8 cores = (batch b, half h of the 4096 query points). BatchNorm uses
global batch stats -> 3 tiny AllReduces of per-channel sum/sumsq.

Layouts (per 128-query tile):
  pixel space: 16 chunks of 512; chunk c = kn*8+g, pixel o = s*16 + q
  (g = query group, q = query-in-group, s = neighbor slot, kn = which knn).
  64-ch activations y1/y2 are packed [128, 4096]: pair u = chunks (2u, 2u+1),
  chunk 2u on partitions 0:64, 2u+1 on 64:128, both at free 512u; conv1/conv2
  use block-diagonal weights so one matmul computes a whole pair.
  y3 is [128, 8192], chunk c at free 512c. Channel-max via DVE stream
  transpose (32x32 blocks) + free-axis reduce; softmax without max-subtract
  (scores bounded, exp(x-4)).

Self-contained: hardcodes shapes; no sibling imports.
"""

import sys

import numpy as np

for _p in ("/opt/trn_rl_repo", "/opt/pypackages"):
    if _p not in sys.path:
        sys.path.append(_p)

import concourse.bass as bass  # noqa: E402  (imported for side effects/typing)
import concourse.mybir as mybir  # noqa: E402
import concourse.tile as tile  # noqa: E402
from concourse import bacc, bass_isa  # noqa: E402
from concourse.bass_utils import run_bass_kernel_spmd  # noqa: E402
from concourse.masks import make_identity  # noqa: E402

F32 = mybir.dt.float32
F16 = mybir.dt.float16
U16 = mybir.dt.uint16
I16 = mybir.dt.int16
AF = mybir.ActivationFunctionType
OP = mybir.AluOpType
AX = mybir.AxisListType

NCORES = 8
B = 4
N = 4096          # candidate points per batch
KNN = 32          # neighbors per knn
QPC = 2048        # query points per core
NT = 16           # query tiles of 128 per core
C1, C2, C3 = 64, 64, 128
NTOT = float(B * N * 2 * KNN)   # BN stat count (global)
BN_EPS = 1e-3
NEG = -1.0e30
EXP_SHIFT = -4.0


def _build_program(single=False):
    nc = bacc.Bacc(
        "TRN2", target_bir_lowering=False, debug=False,
        num_devices=1 if single else NCORES,
    )
    nc._single_core_nocoll = single

    ap = {}
    def din(name, shape, dt=F16):
        ap[name] = nc.dram_tensor(name, shape, dt, kind="ExternalInput").ap()
    din("qf", [11, QPC])
    din("t1", [11, N])
    din("t2", [11, N])
    din("nqsq", [128, NT], F32)
    din("gt", [128, N], F32)
    din("gt2", [128, N], F32)
    din("qr", [4, QPC])
    din("w1b", [8, 128])
    din("w2b", [128, 128])
    din("w3d", [128, 128])
    din("gb1", [C1, 2], F32)
    din("gb2", [C2, 2], F32)
    din("gb3", [C3, 2], F32)
    din("selw", [8, 128])

    ap["out"] = nc.dram_tensor("out", [3, QPC], F32, kind="ExternalOutput").ap()

    ap["y1d"] = nc.dram_tensor("y1d", [NT, 128, 4096], F16).ap()
    ap["y2d"] = nc.dram_tensor("y2d", [NT, 128, 4096], F16).ap()
    ap["y3d"] = nc.dram_tensor("y3d", [NT, 128, 8192], F16).ap()
    ap["g1d"] = nc.dram_tensor("g1d", [NT, 128, 512], F16).ap()
    ap["g2d"] = nc.dram_tensor("g2d", [NT, 128, 512], F16).ap()
    for i, c in ((0, C1), (1, C2), (2, C3)):
        ap[f"arin{i}"] = nc.dram_tensor(f"arin{i}", [c * 2], F32).ap()
        ap[f"arout{i}"] = nc.dram_tensor(f"arout{i}", [c * 2], F32).ap()

    with tile.TileContext(nc) as tc:
        _kernel_body(tc, ap)
    nc.compile()
    return nc


def _kernel_body(tc, d):
    nc = tc.nc
    from contextlib import ExitStack

    ctx = ExitStack()
    with ctx:
        cpool = ctx.enter_context(tc.tile_pool(name="consts", bufs=1))
        w1b = cpool.tile([8, 128], F16)
        w2b = cpool.tile([128, 128], F16)
        w3d = cpool.tile([128, 128], F16)
        gb1 = cpool.tile([C1, 2], F32)
        gb2 = cpool.tile([C2, 2], F32)
        gb3 = cpool.tile([C3, 2], F32)
        selw = cpool.tile([8, 128], F16)
        for nm, sb in [("w1b", w1b), ("w2b", w2b), ("w3d", w3d),
                       ("gb1", gb1), ("gb2", gb2), ("gb3", gb3),
                       ("selw", selw)]:
            nc.sync.dma_start(out=sb[:], in_=d[nm][:])

        spool = ctx.enter_context(tc.tile_pool(name="stats", bufs=1))
        sm1 = spool.tile([128, NT * 8], F32)
        sq1 = spool.tile([128, NT * 8], F32)
        st2 = spool.tile([128, NT * 8 * 6], F32)
        st3 = spool.tile([128, NT * 16 * 6], F32)
        ab1 = spool.tile([128, 2], F32)   # col0 = scale a, col1 = bias b
        ab2 = spool.tile([128, 2], F32)
        ab3 = spool.tile([C3, 2], F32)

        # ---------------- Phase 1: knn + gather + feat + conv1 ----------------
        # 3-stage software pipeline so the Vector queue never stalls:
        # iteration t issues knn(t) | feat(t-1) | conv1(t-2).
        with tc.tile_pool(name="p1c", bufs=1) as c1p, \
             tc.tile_pool(name="p1m", bufs=2) as mpool, \
             tc.tile_pool(name="p1pq", bufs=2, space="PSUM") as pq, \
             tc.tile_pool(name="p1pc", bufs=3, space="PSUM") as pc1, \
             tc.tile_pool(name="p1pt", bufs=2, space="PSUM") as pt, \
             tc.tile_pool(name="p1feat", bufs=1) as fpool, \
             tc.tile_pool(name="p1fp", bufs=2) as fppool, \
             tc.tile_pool(name="p1work", bufs=2) as wp, \
             tc.tile_pool(name="p1y", bufs=2) as yp:
            qf = c1p.tile([11, QPC], F16)
            t1 = c1p.tile([11, N], F16)
            t2 = c1p.tile([11, N], F16)
            nqsq = c1p.tile([128, NT], F32)
            gt = c1p.tile([128, N], F32)
            gt2 = c1p.tile([128, N], F32)
            qr = c1p.tile([4, QPC], F16)
            ident = c1p.tile([128, 128], F16)
            make_identity(nc, ident[:])
            for nm, sb in [("qf", qf), ("t1", t1), ("t2", t2),
                           ("nqsq", nqsq), ("gt", gt), ("gt2", gt2),
                           ("qr", qr)]:
                nc.sync.dma_start(out=sb[:], in_=d[nm][:])

            tiles = [{} for _ in range(NT)]

            def knn_stage(t):
                h = tiles[t]
                vals = wp.tile([128, 64], F32, tag="vals")
                idxu = wp.tile([128, 64], U16, tag="idxu")
                idxi = wp.tile([128, 64], I16, tag="idxi")
                h["vals"], h["idxu"], h["idxi"] = vals, idxu, idxi
                for kn, tab in ((0, t1), (1, t2)):
                    # msb = 2 q.c - |c|^2 - |q|^2 = -d^2 (max == nearest)
                    msb = mpool.tile([128, N], F32, tag="msb")
                    for ch in range(8):
                        pm = pq.tile([128, 512], F32, tag="pm")
                        nc.tensor.matmul(
                            out=pm[:],
                            lhsT=qf[:, t * 128:(t + 1) * 128],
                            rhs=tab[:, ch * 512:(ch + 1) * 512],
                            start=True, stop=True,
                        )
                        nc.scalar.activation(
                            out=msb[:, ch * 512:(ch + 1) * 512], in_=pm[:],
                            func=AF.Identity, bias=nqsq[:, t:t + 1])
                    # top-32 rounds
                    for r in range(4):
                        v8 = vals[:, kn * 32 + r * 8: kn * 32 + r * 8 + 8]
                        i8 = idxu[:, kn * 32 + r * 8: kn * 32 + r * 8 + 8]
                        nc.vector.max(out=v8, in_=msb[:])
                        nc.vector.max_index(out=i8, in_max=v8,
                                            in_values=msb[:])
                        if r < 3:
                            nc.vector.match_replace(
                                out=msb[:], in_to_replace=v8,
                                in_values=msb[:], imm_value=NEG)
                nc.vector.tensor_copy(out=idxi[:], in_=idxu[:])

            def feat_stage(t):
                h = tiles[t]
                vals, idxi = h["vals"], h["idxi"]
                # gather neighbor coords; both tables carry xyz on band rows
                # 16g+{0..2}; convert to fp16 and spill for the fusion phase
                g1 = wp.tile([128, 512], F32, tag="g1")
                g2 = wp.tile([128, 512], F32, tag="g2")
                nc.gpsimd.ap_gather(
                    out_ap=g1[:], in_ap=gt[:], idxs_ap=idxi[:, 0:32],
                    channels=128, num_elems=N, d=1, num_idxs=512)
                nc.gpsimd.ap_gather(
                    out_ap=g2[:], in_ap=gt2[:], idxs_ap=idxi[:, 32:64],
                    channels=128, num_elems=N, d=1, num_idxs=512)
                g1h = wp.tile([128, 512], F16, tag="g1h")
                g2h = wp.tile([128, 512], F16, tag="g2h")
                nc.scalar.activation(out=g1h[:], in_=g1[:], func=AF.Identity)
                nc.scalar.activation(out=g2h[:], in_=g2[:], func=AF.Identity)
                nc.sync.dma_start(out=d["g1d"][t], in_=g1h[:])
                nc.sync.dma_start(out=d["g2d"][t], in_=g2h[:])

                # feat (flat): chunk c at free 512c; band rows -> coord rows
                feat = fpool.tile([4, 8192], F16, tag="feat")
                for g in range(8):
                    nc.scalar.dma_start(
                        out=feat[0:3, g * 512:(g + 1) * 512],
                        in_=g1h[16 * g: 16 * g + 3, :])
                    nc.scalar.dma_start(
                        out=feat[0:3, (8 + g) * 512:(9 + g) * 512],
                        in_=g2h[16 * g: 16 * g + 3, :])

                # dist = sqrt(relu(-vals)) into feat row 3 (pixel layout via
                # PE transpose then per-chunk strided DMAs)
                d2h = wp.tile([128, 64], F16, tag="d2h")
                nc.scalar.activation(out=d2h[:], in_=vals[:], func=AF.Relu,
                                     scale=-1.0)
                nc.scalar.activation(out=d2h[:], in_=d2h[:], func=AF.Sqrt)
                dtp = pt.tile([64, 128], F16, tag="dtp")
                nc.tensor.transpose(out=dtp[:], in_=d2h[:], identity=ident[:])
                d2t = wp.tile([64, 128], F16, tag="d2t")
                nc.scalar.activation(out=d2t[:], in_=dtp[:], func=AF.Identity)
                for kn in (0, 1):
                    for g in range(8):
                        c = kn * 8 + g
                        nc.gpsimd.dma_start(
                            out=feat[3:4, c * 512:(c + 1) * 512]
                                .rearrange("c (s p) -> c s p", s=32),
                            in_=d2t[kn * 32:(kn + 1) * 32,
                                    16 * g:16 * g + 16])

                # resi = nn - q (in place on coord rows; gpsimd -> off the
                # DVE critical path)
                qrt = qr[0:3, t * 128:(t + 1) * 128]
                for kn in (0, 1):
                    nc.gpsimd.tensor_tensor(
                        out=feat[0:3, kn * 4096:(kn + 1) * 4096]
                            .rearrange("c (g s p) -> c g s p", g=8, s=32),
                        in0=feat[0:3, kn * 4096:(kn + 1) * 4096]
                            .rearrange("c (g s p) -> c g s p", g=8, s=32),
                        in1=qrt.rearrange("c (g p) -> c g p", g=8)
                            .unsqueeze(2).to_broadcast([3, 8, 32, 16]),
                        op=OP.subtract)

                # fold chunks into pairs: featp rows 0:4 = even chunk,
                # rows 4:8 = odd chunk, pair u at free 512u
                featp = fppool.tile([8, 4096], F16, tag="featp")
                h["featp"] = featp
                nc.gpsimd.dma_start(
                    out=featp[0:4, :].rearrange("c (u f) -> c u f", u=8),
                    in_=feat[0:4, :].rearrange("c (u f) -> c u f", u=8)
                        [:, :, 0:512])
                nc.gpsimd.dma_start(
                    out=featp[4:8, :].rearrange("c (u f) -> c u f", u=8),
                    in_=feat[0:4, :].rearrange("c (u f) -> c u f", u=8)
                        [:, :, 512:1024])

            def conv_stage(t):
                h = tiles[t]
                featp = h["featp"]
                # conv1: 8 paired matmuls -> y1 packed [128, 4096]
                y1 = yp.tile([128, 4096], F16, tag="y1")
                for u in range(8):
                    pm1 = pc1.tile([128, 512], F32, tag="pm1")
                    nc.tensor.matmul(
                        out=pm1[:], lhsT=w1b[:],
                        rhs=featp[:, u * 512:(u + 1) * 512],
                        start=True, stop=True)
                    # stats on the scalar engine (P1's DVE is saturated by
                    # the topk): sum rides the copy, sumsq via Square
                    s_ = t * 8 + u
                    nc.scalar.activation(
                        out=y1[:, u * 512:(u + 1) * 512], in_=pm1[:],
                        func=AF.Identity, accum_out=sm1[:, s_:s_ + 1])
                    sqs = wp.tile([128, 512], F16, tag="sqs")
                    nc.scalar.activation(
                        out=sqs[:], in_=pm1[:], func=AF.Square,
                        accum_out=sq1[:, s_:s_ + 1])
                nc.sync.dma_start(out=d["y1d"][t], in_=y1[:])
                h.clear()

            for t in range(NT):
                knn_stage(t)
                if t >= 1:
                    feat_stage(t - 1)
                if t >= 2:
                    conv_stage(t - 2)
            feat_stage(NT - 1)
            conv_stage(NT - 2)
            conv_stage(NT - 1)

        _bn_fold_raw(tc, 0, sm1, sq1, gb1, ab1, d["arin0"], d["arout0"], C1)

        # ---------------- Phase 2: apply BN1+relu, conv2 ----------------
        with tc.tile_pool(name="p2y", bufs=2) as yp, \
             tc.tile_pool(name="p2psum", bufs=6, space="PSUM") as cp:
            for t in range(NT):
                y1 = yp.tile([128, 4096], F16, tag="y1l")
                nc.sync.dma_start(out=y1[:], in_=d["y1d"][t])
                nc.scalar.activation(
                    out=y1[:], in_=y1[:], func=AF.Relu,
                    scale=ab1[:, 0:1], bias=ab1[:, 1:2])
                y2 = yp.tile([128, 4096], F16, tag="y2")
                for u in range(8):
                    pm = cp.tile([128, 512], F32, tag="pm2")
                    nc.tensor.matmul(
                        out=pm[:], lhsT=w2b[:],
                        rhs=y1[:, u * 512:(u + 1) * 512],
                        start=True, stop=True)
                    nc.vector.bn_stats(
                        out=st2[:, (t * 8 + u) * 6:(t * 8 + u + 1) * 6],
                        in_=pm[:])
                    # 5 copies on scalar, 3 on vector: P2 is scalar-bound
                    if u % 3 != 2:
                        nc.scalar.activation(
                            out=y2[:, u * 512:(u + 1) * 512], in_=pm[:],
                            func=AF.Identity)
                    else:
                        nc.vector.tensor_copy(
                            out=y2[:, u * 512:(u + 1) * 512], in_=pm[:])
                nc.sync.dma_start(out=d["y2d"][t], in_=y2[:])

        _bn_fold(tc, 1, st2, gb2, ab2, d["arin1"], d["arout1"], C2)

        # ---------------- Phase 3: apply BN2+relu, conv3 ----------------
        with tc.tile_pool(name="p3y", bufs=2) as yp, \
             tc.tile_pool(name="p3psum", bufs=6, space="PSUM") as cp:
            for t in range(NT):
                y2 = yp.tile([128, 4096], F16, tag="y2l")
                nc.sync.dma_start(out=y2[:], in_=d["y2d"][t])
                nc.scalar.activation(
                    out=y2[:], in_=y2[:], func=AF.Relu,
                    scale=ab2[:, 0:1], bias=ab2[:, 1:2])
                y3 = yp.tile([128, 8192], F16, tag="y3")
                for c in range(16):
                    bp_ = 64 * (c % 2)
                    pm = cp.tile([C3, 512], F32, tag="pm3")
                    nc.tensor.matmul(
                        out=pm[:], lhsT=w3d[bp_:bp_ + 64, :],
                        rhs=y2[bp_:bp_ + 64,
                               512 * (c // 2):512 * (c // 2) + 512],
                        start=True, stop=True)
                    nc.vector.bn_stats(
                        out=st3[:, (t * 16 + c) * 6:(t * 16 + c + 1) * 6],
                        in_=pm[:])
                    # 12 copies on scalar, 4 on vector: balances S vs the
                    # bn_stats-loaded DVE
                    if c % 4 != 3:
                        nc.scalar.activation(
                            out=y3[:, c * 512:(c + 1) * 512], in_=pm[:],
                            func=AF.Identity)
                    else:
                        nc.vector.tensor_copy(
                            out=y3[:, c * 512:(c + 1) * 512], in_=pm[:])
                nc.sync.dma_start(out=d["y3d"][t], in_=y3[:])

        _bn_fold(tc, 2, st3, gb3, ab3, d["arin2"], d["arout2"], C3)

        # ------------- Phase 4: scores, softmax, fusion, output -------------
        with tc.tile_pool(name="p4z", bufs=2) as zp, \
             tc.tile_pool(name="p4zw", bufs=2) as zw, \
             tc.tile_pool(name="p4work", bufs=2) as wp, \
             tc.tile_pool(name="p4psum", bufs=2, space="PSUM") as pp4, \
             tc.tile_pool(name="p4out", bufs=1) as op_:
            outsb = op_.tile([4, QPC], F32)
            for t in range(NT):
                z = zp.tile([128, 8192], F16, tag="z")
                nc.sync.dma_start(out=z[:], in_=d["y3d"][t])
                # per-channel affine a3*z + b3 (relu deferred past the max)
                nc.scalar.activation(
                    out=z[:], in_=z[:], func=AF.Identity,
                    scale=ab3[:, 0:1], bias=ab3[:, 1:2])
                # channel max: stream-transpose 32x32 blocks first, reduce
                # each block over free (split DVE/GPSIMD), then fold the 4
                # partition groups on small [*, 256] tiles via DMA shifts
                # (engines need same-start-partition operands)
                zT = zw.tile([128, 8192], F16, tag="zT")
                nc.vector.transpose(out=zT[:], in_=z[:])
                R = wp.tile([128, 256], F16, tag="R")
                nc.vector.tensor_reduce(
                    out=R[:],
                    in_=zT[:].rearrange("c (j e) -> c j e", e=32),
                    axis=AX.X, op=OP.max)
                Rs = wp.tile([64, 256], F16, tag="Rs")
                nc.scalar.dma_start(out=Rs[:], in_=R[64:128, :])
                R2 = wp.tile([64, 256], F16, tag="R2")
                nc.vector.tensor_tensor(out=R2[:], in0=R[0:64, :],
                                        in1=Rs[:], op=OP.max)
                R2s = wp.tile([32, 256], F16, tag="R2s")
                nc.scalar.dma_start(out=R2s[:], in_=R2[32:64, :])
                T32 = wp.tile([32, 256], F16, tag="T32")
                nc.vector.tensor_tensor(out=T32[:], in0=R2[0:32, :],
                                        in1=R2s[:], op=OP.max)
                # relu (commutes with the channel max)
                nc.vector.tensor_scalar_max(T32[:], T32[:], 0.0)
                # assemble raw scores per knn half in q-major layout:
                # scX[g, q*32 + sl*16 + sh] <- T32[16*sl+q, 16*(kn*8+g)+sh]
                # (slot s = 2*sh + sl)
                scA = wp.tile([8, 512], F16, tag="scA")
                scB = wp.tile([8, 512], F16, tag="scB")
                for kn, sct in ((0, scA), (1, scB)):
                    for g in range(8):
                        cbase = 16 * (kn * 8 + g)
                        ov = sct[g:g + 1, :].rearrange(
                            "c (q sl sh) -> c sl q sh", q=16, sl=2)
                        for sl in (0, 1):
                            eng = nc.sync if (g + sl) % 2 == 0 else nc.gpsimd
                            eng.dma_start(
                                out=ov[:, sl],
                                in_=T32[16 * sl:16 * sl + 16,
                                        cbase:cbase + 16])
                # per-query max over the 64 slots, subtract, exponentiate
                qmA = wp.tile([8, 16], F16, tag="qmA")
                qmB = wp.tile([8, 16], F16, tag="qmB")
                for sct, qm in ((scA, qmA), (scB, qmB)):
                    nc.vector.tensor_reduce(
                        out=qm[:],
                        in_=sct[:].rearrange("c (q z) -> c q z", z=32),
                        axis=AX.X, op=OP.max)
                nc.vector.tensor_tensor(out=qmA[:], in0=qmA[:], in1=qmB[:],
                                        op=OP.max)
                for sct in (scA, scB):
                    nc.vector.tensor_tensor(
                        out=sct[:].rearrange("c (q z) -> c q z", z=32),
                        in0=sct[:].rearrange("c (q z) -> c q z", z=32),
                        in1=qmA[:].unsqueeze(2).to_broadcast([8, 16, 32]),
                        op=OP.subtract)
                    nc.scalar.activation(out=sct[:], in_=sct[:], func=AF.Exp)
                # denominators over the 64 slots of each query
                qsA = wp.tile([8, 16], F32, tag="qsA")
                qsB = wp.tile([8, 16], F32, tag="qsB")
                for sct, qs in ((scA, qsA), (scB, qsB)):
                    nc.vector.tensor_reduce(
                        out=qs[:],
                        in_=sct[:].rearrange("c (q z) -> c q z", z=32),
                        axis=AX.X, op=OP.add)
                nc.vector.tensor_tensor(out=qsA[:], in0=qsA[:], in1=qsB[:],
                                        op=OP.add)
                nc.vector.reciprocal(out=qsA[:], in_=qsA[:])
                rec = wp.tile([8, 16], F16, tag="rec")
                nc.vector.tensor_copy(out=rec[:], in_=qsA[:])
                for sct in (scA, scB):
                    nc.vector.tensor_tensor(
                        out=sct[:].rearrange("c (q z) -> c q z", z=32),
                        in0=sct[:].rearrange("c (q z) -> c q z", z=32),
                        in1=rec[:].unsqueeze(2).to_broadcast([8, 16, 32]),
                        op=OP.mult)
                # replicate weight rows onto band partitions, multiply with
                # raw coords, segment-reduce over slots
                wr1 = wp.tile([128, 512], F16, tag="wr1")
                wr2 = wp.tile([128, 512], F16, tag="wr2")
                for sct, wr in ((scA, wr1), (scB, wr2)):
                    pw = pp4.tile([128, 512], F32, tag="pw")
                    nc.tensor.matmul(
                        out=pw[:], lhsT=selw[:], rhs=sct[:],
                        start=True, stop=True)
                    nc.scalar.activation(out=wr[:], in_=pw[:],
                                         func=AF.Identity)
                # coords are in o = s*16+q layout -> view them q-major to
                # line up with wr (q-major from the selector matmul)
                g1l = wp.tile([128, 512], F16, tag="g1l")
                g2l = wp.tile([128, 512], F16, tag="g2l")
                nc.sync.dma_start(out=g1l[:], in_=d["g1d"][t])
                nc.sync.dma_start(out=g2l[:], in_=d["g2d"][t])
                pr = wp.tile([128, 512], F16, tag="pr")
                gv1 = g1l[:].rearrange("c (sh sl q) -> c q sl sh",
                                       sh=16, sl=2)
                gv2 = g2l[:].rearrange("c (sh sl q) -> c q sl sh",
                                       sh=16, sl=2)
                wv1 = wr1[:].rearrange("c (q sl sh) -> c q sl sh",
                                       q=16, sl=2)
                wv2 = wr2[:].rearrange("c (q sl sh) -> c q sl sh",
                                       q=16, sl=2)
                pv = pr[:].rearrange("c (q sl sh) -> c q sl sh", q=16, sl=2)
                nc.vector.tensor_tensor(out=pv, in0=gv1, in1=wv1, op=OP.mult)
                nc.gpsimd.tensor_tensor(out=wv2, in0=gv2, in1=wv2,
                                        op=OP.mult)
                nc.vector.tensor_tensor(out=pr[:], in0=pr[:], in1=wr2[:],
                                        op=OP.add)
                fp_ = wp.tile([128, 16], F32, tag="fp")
                nc.vector.tensor_reduce(
                    out=fp_[:], in_=pr[:].rearrange("c (q z) -> c q z", z=32),
                    axis=AX.X, op=OP.add)
                for g in range(8):
                    nc.scalar.dma_start(
                        out=outsb[0:3,
                                  t * 128 + 16 * g: t * 128 + 16 * g + 16],
                        in_=fp_[16 * g: 16 * g + 3, :])
            nc.sync.dma_start(out=d["out"][:], in_=outsb[0:3, :])


def _bn_fold_raw(tc, li, sm, sq, gbe, ab, arin, arout, C):
    """Like _bn_fold but from raw per-slot (sum, sumsq) accumulators."""
    nc = tc.nc
    ntot = NTOT / (NCORES if getattr(nc, "_single_core_nocoll", False) else 1)
    with tc.tile_pool(name=f"bnr{li}", bufs=1) as bp:
        ss = bp.tile([128, 2], F32)
        nc.vector.tensor_reduce(out=ss[:, 0:1], in_=sm[:], axis=AX.X,
                                op=OP.add)
        nc.vector.tensor_reduce(out=ss[:, 1:2], in_=sq[:], axis=AX.X,
                                op=OP.add)
        sh = bp.tile([C, 2], F32)
        nc.sync.dma_start(out=sh[:], in_=ss[C:2 * C, :])
        sc = bp.tile([C, 2], F32)
        nc.vector.tensor_tensor(out=sc[:], in0=ss[0:C, :], in1=sh[:],
                                op=OP.add)
        nc.sync.dma_start(out=arin[:], in_=sc[:])
        if getattr(nc, "_single_core_nocoll", False):
            nc.sync.dma_start(out=arout[:], in_=arin[:])
        else:
            nc.gpsimd.collective_compute(
                "AllReduce", OP.add, replica_groups=[list(range(NCORES))],
                ins=[arin.opt()], outs=[arout.opt()])
        ar = bp.tile([C, 2], F32)
        nc.sync.dma_start(out=ar[:], in_=arout[:])
        mean = bp.tile([C, 1], F32)
        var = bp.tile([C, 1], F32)
        nc.vector.tensor_scalar_mul(mean[:], ar[:, 0:1], 1.0 / ntot)
        nc.vector.tensor_scalar_mul(var[:], ar[:, 1:2], 1.0 / ntot)
        mm = bp.tile([C, 1], F32)
        nc.vector.tensor_tensor(out=mm[:], in0=mean[:], in1=mean[:],
                                op=OP.mult)
        nc.vector.tensor_tensor(out=var[:], in0=var[:], in1=mm[:],
                                op=OP.subtract)
        nc.vector.tensor_scalar_add(var[:], var[:], BN_EPS)
        nc.scalar.activation(out=var[:], in_=var[:], func=AF.Sqrt)
        nc.vector.reciprocal(out=var[:], in_=var[:])  # rsqrt(var+eps)
        nc.vector.tensor_tensor(out=ab[0:C, 0:1], in0=var[:],
                                in1=gbe[:, 0:1], op=OP.mult)     # a
        nc.vector.tensor_tensor(out=mm[:], in0=ab[0:C, 0:1], in1=mean[:],
                                op=OP.mult)
        nc.vector.tensor_tensor(out=ab[0:C, 1:2], in0=gbe[:, 1:2], in1=mm[:],
                                op=OP.subtract)       # b = be - a*mean
        nc.vector.tensor_copy(out=ab[C:2 * C, :], in_=ab[0:C, :])


def _bn_fold(tc, li, st, gbe, ab, arin, arout, C):
    """bn_aggr per partition, convert to (sum, sumsq), fold dup halves for
    64-ch layers, AllReduce, then a = g*rsqrt(var+eps), b = be - a*mean."""
    nc = tc.nc
    n_loc = float(QPC * 64 * C // 128)  # pixels per partition slot
    ntot = NTOT / (NCORES if getattr(nc, "_single_core_nocoll", False) else 1)
    with tc.tile_pool(name=f"bn{li}", bufs=1) as bp:
        ag = bp.tile([128, 2], F32)
        nc.vector.bn_aggr(out=ag[:], in_=st[:])
        ss = bp.tile([128, 2], F32)
        m2 = bp.tile([128, 1], F32)
        nc.vector.tensor_tensor(out=m2[:], in0=ag[:, 0:1], in1=ag[:, 0:1],
                                op=OP.mult)
        nc.vector.tensor_tensor(out=ss[:, 1:2], in0=ag[:, 1:2], in1=m2[:],
                                op=OP.add)            # var + mean^2
        nc.vector.tensor_scalar_mul(ss[:, 1:2], ss[:, 1:2], n_loc)
        nc.vector.tensor_scalar_mul(ss[:, 0:1], ag[:, 0:1], n_loc)
        if C == 64:
            sh = bp.tile([64, 2], F32)
            nc.sync.dma_start(out=sh[:], in_=ss[64:128, :])
            sc = bp.tile([64, 2], F32)
            nc.vector.tensor_tensor(out=sc[:], in0=ss[0:64, :],
                                    in1=sh[:], op=OP.add)
        else:
            sc = ss
        nc.sync.dma_start(out=arin[:], in_=sc[:])
        if getattr(nc, "_single_core_nocoll", False):
            nc.sync.dma_start(out=arout[:], in_=arin[:])
        else:
            nc.gpsimd.collective_compute(
                "AllReduce", OP.add, replica_groups=[list(range(NCORES))],
                ins=[arin.opt()], outs=[arout.opt()])
        ar = bp.tile([C, 2], F32)
        nc.sync.dma_start(out=ar[:], in_=arout[:])
        mean = bp.tile([C, 1], F32)
        var = bp.tile([C, 1], F32)
        nc.vector.tensor_scalar_mul(mean[:], ar[:, 0:1], 1.0 / ntot)
        nc.vector.tensor_scalar_mul(var[:], ar[:, 1:2], 1.0 / ntot)
        mm = bp.tile([C, 1], F32)
        nc.vector.tensor_tensor(out=mm[:], in0=mean[:], in1=mean[:],
                                op=OP.mult)
        nc.vector.tensor_tensor(out=var[:], in0=var[:], in1=mm[:],
                                op=OP.subtract)
        nc.vector.tensor_scalar_add(var[:], var[:], BN_EPS)
        nc.scalar.activation(out=var[:], in_=var[:], func=AF.Sqrt)
        nc.vector.reciprocal(out=var[:], in_=var[:])  # rsqrt(var+eps)
        nc.vector.tensor_tensor(out=ab[0:C, 0:1], in0=var[:],
                                in1=gbe[:, 0:1], op=OP.mult)     # a
        nc.vector.tensor_tensor(out=mm[:], in0=ab[0:C, 0:1], in1=mean[:],
                                op=OP.mult)
        nc.vector.tensor_tensor(out=ab[0:C, 1:2], in0=gbe[:, 1:2], in1=mm[:],
                                op=OP.subtract)       # b = be - a*mean
        if C == 64:
            nc.vector.tensor_copy(out=ab[C:2 * C, :], in_=ab[0:C, :])


_PROGRAM = None
LAST_RESULT = None


def _get_program():
    global _PROGRAM
    if _PROGRAM is None:
        _PROGRAM = _build_program()
    return _PROGRAM


def _split16(x):
    hi = x.astype(np.float16).astype(np.float32)
    return hi, (x - hi).astype(np.float32)


def _prep_core_inputs(points1, points2, W1, W2, W3, gs, bes, b, h):
    p1 = points1[b]          # [3, N]
    p2 = points2[b]
    q = p1[:, h * QPC:(h + 1) * QPC]            # [3, QPC]

    qhi, qlo = _split16(q)
    qf = np.concatenate([2.0 * qhi, 2.0 * qhi, 2.0 * qlo,
                         np.ones((2, QPC), np.float32)], axis=0)

    def cand_tab(p):
        chi, clo = _split16(p)
        csq = (p * p).sum(axis=0)
        cshi, cslo = _split16(csq)
        return np.concatenate([chi, clo, chi, -cshi[None], -cslo[None]],
                              axis=0).astype(np.float16)   # [11, N]

    gtab = np.zeros((128, N), np.float32)
    gtab2 = np.zeros((128, N), np.float32)
    for g in range(8):
        gtab[16 * g + 0:16 * g + 3] = p1
        gtab2[16 * g + 0:16 * g + 3] = p2
    qraw = np.zeros((4, QPC), np.float16)
    qraw[0:3] = q.astype(np.float16)
    nqsqv = -(q * q).sum(axis=0).reshape(NT, 128).T.astype(np.float32)

    w1t = np.ascontiguousarray(W1.T).astype(np.float16)    # [4, 64]
    w2t = np.ascontiguousarray(W2.T).astype(np.float16)    # [64, 64]
    w3t = np.ascontiguousarray(W3.T).astype(np.float16)    # [64, 128]
    w1blk = np.zeros((8, 128), np.float16)
    w1blk[0:4, 0:64] = w1t
    w1blk[4:8, 64:128] = w1t
    w2blk = np.zeros((128, 128), np.float16)
    w2blk[0:64, 0:64] = w2t
    w2blk[64:128, 64:128] = w2t
    w3dup = np.concatenate([w3t, w3t], axis=0).astype(np.float16)

    selw = np.zeros((8, 128), np.float16)
    for g in range(8):
        for c3 in range(3):
            selw[g, 16 * g + c3] = 1.0

    return {
        "qf": qf.astype(np.float16),
        "t1": cand_tab(p1), "t2": cand_tab(p2),
        "nqsq": np.ascontiguousarray(nqsqv),
        "gt": gtab, "gt2": gtab2, "qr": qraw,
        "w1b": w1blk, "w2b": w2blk, "w3d": w3dup,
        "gb1": np.stack([gs[0], bes[0]], axis=1).astype(np.float32),
        "gb2": np.stack([gs[1], bes[1]], axis=1).astype(np.float32),
        "gb3": np.stack([gs[2], bes[2]], axis=1).astype(np.float32),
        "selw": selw,
    }


def kernel(points1, points2, k, t, W1, b1, g1, be1, W2, b2, g2, be2,
           W3, b3, g3, be3):
    # b1/b2/b3 cancel inside train-mode BatchNorm; t is unused by the net.
    assert int(np.asarray(k)) == KNN
    points1 = np.asarray(points1, np.float32)
    points2 = np.asarray(points2, np.float32)
    gs = [np.asarray(g1, np.float32), np.asarray(g2, np.float32),
          np.asarray(g3, np.float32)]
    bes = [np.asarray(be1, np.float32), np.asarray(be2, np.float32),
           np.asarray(be3, np.float32)]
    Ws = [np.asarray(W1, np.float32), np.asarray(W2, np.float32),
          np.asarray(W3, np.float32)]

    in_maps = []
    for c in range(NCORES):
        b, h = divmod(c, 2)
        in_maps.append(_prep_core_inputs(points1, points2, *Ws, gs, bes, b, h))

    nc = _get_program()
    bkr = run_bass_kernel_spmd(nc, in_maps, list(range(NCORES)))
    global LAST_RESULT
    LAST_RESULT = bkr
    res = bkr.results

    out = np.zeros((B, 3, N), np.float32)
    for c in range(NCORES):
        b, h = divmod(c, 2)
        out[b, :, h * QPC:(h + 1) * QPC] = res[c]["out"]
    return out


# revision 43
# speedup vs baseline: 1.0672x; 1.0015x over previous
"""PointsFusion Trainium2 kernel (fp16 fast path).

Pipeline per batch b (B=4, N=4096, k=32):
  knn1 = 32-NN of p1 in p1, knn2 = 32-NN of p1 in p2 (exact, DVE 8-max rounds
  on fp32 recentred scores; dist matmul uses fp16 split-channel inputs so the
  fp32 PSUM result is exact to ~1e-6)
  gather neighbor coords, features (resi, dist) -> conv(4->64)->BN->relu
  -> conv(64->64)->BN->relu -> conv(64->128)->BN->relu -> channel-max scores
  -> softmax over 64 neighbors -> weighted sum of neighbor coords.

Sharding: 8 cores = (batch b, half h of the 4096 query points). BatchNorm uses
global batch stats -> 3 tiny AllReduces of per-channel sum/sumsq.

Layouts (per 128-query tile):
  pixel space: 16 chunks of 512; chunk c = kn*8+g, pixel o = s*16 + q
  (g = query group, q = query-in-group, s = neighbor slot, kn = which knn).
  64-ch activations y1/y2 are packed [128, 4096]: pair u = chunks (2u, 2u+1),
  chunk 2u on partitions 0:64, 2u+1 on 64:128, both at free 512u; conv1/conv2
  use block-diagonal weights so one matmul computes a whole pair.
  y3 is [128, 8192], chunk c at free 512c. Channel-max via DVE stream
  transpose (32x32 blocks) + free-axis reduce; softmax without max-subtract
  (scores bounded, exp(x-4)).

Self-contained: hardcodes shapes; no sibling imports.
"""

import sys

import numpy as np

for _p in ("/opt/trn_rl_repo", "/opt/pypackages"):
    if _p not in sys.path:
        sys.path.append(_p)

import concourse.bass as bass  # noqa: E402  (imported for side effects/typing)
import concourse.mybir as mybir  # noqa: E402
import concourse.tile as tile  # noqa: E402
from concourse import bacc, bass_isa  # noqa: E402
from concourse.bass_utils import run_bass_kernel_spmd  # noqa: E402
from concourse.masks import make_identity  # noqa: E402

F32 = mybir.dt.float32
F16 = mybir.dt.float16
U16 = mybir.dt.uint16
I16 = mybir.dt.int16
AF = mybir.ActivationFunctionType
OP = mybir.AluOpType
AX = mybir.AxisListType

NCORES = 8
B = 4
N = 4096          # candidate points per batch
KNN = 32          # neighbors per knn
QPC = 2048        # query points per core
NT = 16           # query tiles of 128 per core
C1, C2, C3 = 64, 64, 128
NTOT = float(B * N * 2 * KNN)   # BN stat count (global)
BN_EPS = 1e-3
NEG = -1.0e30
EXP_SHIFT = -4.0


def _build_program(single=False):
    nc = bacc.Bacc(
        "TRN2", target_bir_lowering=False, debug=False,
        num_devices=1 if single else NCORES,
    )
    nc._single_core_nocoll = single

    ap = {}
    def din(name, shape, dt=F16):
        ap[name] = nc.dram_tensor(name, shape, dt, kind="ExternalInput").ap()
    din("qf", [11, QPC])
    din("t1", [11, N])
    din("t2", [11, N])
    din("nqsq", [128, NT], F32)
    din("gt", [128, N], F32)
    din("gt2", [128, N], F32)
    din("qr", [4, QPC])
    din("w1b", [8, 128])
    din("w2b", [128, 128])
    din("w3d", [128, 128])
    din("gb1", [C1, 2], F32)
    din("gb2", [C2, 2], F32)
    din("gb3", [C3, 2], F32)
    din("selw", [8, 128])

    ap["out"] = nc.dram_tensor("out", [3, QPC], F32, kind="ExternalOutput").ap()

    ap["y1d"] = nc.dram_tensor("y1d", [NT, 128, 4096], F16).ap()
    ap["y2d"] = nc.dram_tensor("y2d", [NT, 128, 4096], F16).ap()
    ap["y3d"] = nc.dram_tensor("y3d", [NT, 128, 8192], F16).ap()
    ap["g1d"] = nc.dram_tensor("g1d", [NT, 128, 512], F16).ap()
    ap["g2d"] = nc.dram_tensor("g2d", [NT, 128, 512], F16).ap()
    for i, c in ((0, C1), (1, C2), (2, C3)):
        ap[f"arin{i}"] = nc.dram_tensor(f"arin{i}", [c * 2], F32).ap()
        ap[f"arout{i}"] = nc.dram_tensor(f"arout{i}", [c * 2], F32).ap()

    with tile.TileContext(nc) as tc:
        _kernel_body(tc, ap)
    nc.compile()
    return nc


def _kernel_body(tc, d):
    nc = tc.nc
    from contextlib import ExitStack

    ctx = ExitStack()
    with ctx:
        cpool = ctx.enter_context(tc.tile_pool(name="consts", bufs=1))
        w1b = cpool.tile([8, 128], F16)
        w2b = cpool.tile([128, 128], F16)
        w3d = cpool.tile([128, 128], F16)
        gb1 = cpool.tile([C1, 2], F32)
        gb2 = cpool.tile([C2, 2], F32)
        gb3 = cpool.tile([C3, 2], F32)
        selw = cpool.tile([8, 128], F16)
        for nm, sb in [("w1b", w1b), ("w2b", w2b), ("w3d", w3d),
                       ("gb1", gb1), ("gb2", gb2), ("gb3", gb3),
                       ("selw", selw)]:
            nc.sync.dma_start(out=sb[:], in_=d[nm][:])

        spool = ctx.enter_context(tc.tile_pool(name="stats", bufs=1))
        sm1 = spool.tile([128, NT * 8], F32)
        sq1 = spool.tile([128, NT * 8], F32)
        st2 = spool.tile([128, NT * 8 * 6], F32)
        st3 = spool.tile([128, NT * 16 * 6], F32)
        ab1 = spool.tile([128, 2], F32)   # col0 = scale a, col1 = bias b
        ab2 = spool.tile([128, 2], F32)
        ab3 = spool.tile([C3, 2], F32)

        # ---------------- Phase 1: knn + gather + feat + conv1 ----------------
        # 3-stage software pipeline so the Vector queue never stalls:
        # iteration t issues knn(t) | feat(t-1) | conv1(t-2).
        with tc.tile_pool(name="p1c", bufs=1) as c1p, \
             tc.tile_pool(name="p1m", bufs=2) as mpool, \
             tc.tile_pool(name="p1pq", bufs=2, space="PSUM") as pq, \
             tc.tile_pool(name="p1pc", bufs=3, space="PSUM") as pc1, \
             tc.tile_pool(name="p1pt", bufs=2, space="PSUM") as pt, \
             tc.tile_pool(name="p1feat", bufs=1) as fpool, \
             tc.tile_pool(name="p1fp", bufs=2) as fppool, \
             tc.tile_pool(name="p1work", bufs=2) as wp, \
             tc.tile_pool(name="p1y", bufs=2) as yp:
            qf = c1p.tile([11, QPC], F16)
            t1 = c1p.tile([11, N], F16)
            t2 = c1p.tile([11, N], F16)
            nqsq = c1p.tile([128, NT], F32)
            gt = c1p.tile([128, N], F32)
            gt2 = c1p.tile([128, N], F32)
            qr = c1p.tile([4, QPC], F16)
            ident = c1p.tile([128, 128], F16)
            make_identity(nc, ident[:])
            for nm, sb in [("qf", qf), ("t1", t1), ("t2", t2),
                           ("nqsq", nqsq), ("gt", gt), ("gt2", gt2),
                           ("qr", qr)]:
                nc.sync.dma_start(out=sb[:], in_=d[nm][:])

            tiles = [{} for _ in range(NT)]

            def knn_stage(t):
                h = tiles[t]
                vals = wp.tile([128, 64], F32, tag="vals")
                idxu = wp.tile([128, 64], U16, tag="idxu")
                idxi = wp.tile([128, 64], I16, tag="idxi")
                h["vals"], h["idxu"], h["idxi"] = vals, idxu, idxi
                # msb = 2 q.c - |c|^2 - |q|^2 = -d^2 (max == nearest)
                msbs = []
                for kn, tab in ((0, t1), (1, t2)):
                    msb = mpool.tile([128, N], F32, tag="msb")
                    for ch in range(8):
                        pm = pq.tile([128, 512], F32, tag="pm")
                        nc.tensor.matmul(
                            out=pm[:],
                            lhsT=qf[:, t * 128:(t + 1) * 128],
                            rhs=tab[:, ch * 512:(ch + 1) * 512],
                            start=True, stop=True,
                        )
                        nc.scalar.activation(
                            out=msb[:, ch * 512:(ch + 1) * 512], in_=pm[:],
                            func=AF.Identity, bias=nqsq[:, t:t + 1])
                    msbs.append(msb)
                # top-32 rounds, the two knn streams interleaved op-by-op
                # so each stream's intra-round latency hides under the other
                def sl(kn, r):
                    o = kn * 32 + r * 8
                    return vals[:, o:o + 8], idxu[:, o:o + 8]
                for r in range(4):
                    for kn in (0, 1):
                        v8, _ = sl(kn, r)
                        nc.vector.max(out=v8, in_=msbs[kn][:])
                    for kn in (0, 1):
                        v8, i8 = sl(kn, r)
                        nc.vector.max_index(out=i8, in_max=v8,
                                            in_values=msbs[kn][:])
                    if r < 3:
                        for kn in (0, 1):
                            v8, _ = sl(kn, r)
                            nc.vector.match_replace(
                                out=msbs[kn][:], in_to_replace=v8,
                                in_values=msbs[kn][:], imm_value=NEG)
                nc.vector.tensor_copy(out=idxi[:], in_=idxu[:])

            def feat_stage(t):
                h = tiles[t]
                vals, idxi = h["vals"], h["idxi"]
                # gather neighbor coords; both tables carry xyz on band rows
                # 16g+{0..2}; convert to fp16 and spill for the fusion phase
                g1 = wp.tile([128, 512], F32, tag="g1")
                g2 = wp.tile([128, 512], F32, tag="g2")
                nc.gpsimd.ap_gather(
                    out_ap=g1[:], in_ap=gt[:], idxs_ap=idxi[:, 0:32],
                    channels=128, num_elems=N, d=1, num_idxs=512)
                nc.gpsimd.ap_gather(
                    out_ap=g2[:], in_ap=gt2[:], idxs_ap=idxi[:, 32:64],
                    channels=128, num_elems=N, d=1, num_idxs=512)
                g1h = wp.tile([128, 512], F16, tag="g1h")
                g2h = wp.tile([128, 512], F16, tag="g2h")
                nc.scalar.activation(out=g1h[:], in_=g1[:], func=AF.Identity)
                nc.scalar.activation(out=g2h[:], in_=g2[:], func=AF.Identity)
                nc.sync.dma_start(out=d["g1d"][t], in_=g1h[:])
                nc.sync.dma_start(out=d["g2d"][t], in_=g2h[:])

                # feat (flat): chunk c at free 512c; band rows -> coord rows
                feat = fpool.tile([4, 8192], F16, tag="feat")
                for g in range(8):
                    nc.scalar.dma_start(
                        out=feat[0:3, g * 512:(g + 1) * 512],
                        in_=g1h[16 * g: 16 * g + 3, :])
                    nc.scalar.dma_start(
                        out=feat[0:3, (8 + g) * 512:(9 + g) * 512],
                        in_=g2h[16 * g: 16 * g + 3, :])

                # dist = sqrt(relu(-vals)) into feat row 3 (pixel layout via
                # PE transpose then per-chunk strided DMAs)
                d2h = wp.tile([128, 64], F16, tag="d2h")
                nc.scalar.activation(out=d2h[:], in_=vals[:], func=AF.Relu,
                                     scale=-1.0)
                nc.scalar.activation(out=d2h[:], in_=d2h[:], func=AF.Sqrt)
                dtp = pt.tile([64, 128], F16, tag="dtp")
                nc.tensor.transpose(out=dtp[:], in_=d2h[:], identity=ident[:])
                d2t = wp.tile([64, 128], F16, tag="d2t")
                nc.scalar.activation(out=d2t[:], in_=dtp[:], func=AF.Identity)
                for kn in (0, 1):
                    for g in range(8):
                        c = kn * 8 + g
                        nc.gpsimd.dma_start(
                            out=feat[3:4, c * 512:(c + 1) * 512]
                                .rearrange("c (s p) -> c s p", s=32),
                            in_=d2t[kn * 32:(kn + 1) * 32,
                                    16 * g:16 * g + 16])

                # resi = nn - q (in place on coord rows; gpsimd -> off the
                # DVE critical path)
                qrt = qr[0:3, t * 128:(t + 1) * 128]
                for kn in (0, 1):
                    nc.gpsimd.tensor_tensor(
                        out=feat[0:3, kn * 4096:(kn + 1) * 4096]
                            .rearrange("c (g s p) -> c g s p", g=8, s=32),
                        in0=feat[0:3, kn * 4096:(kn + 1) * 4096]
                            .rearrange("c (g s p) -> c g s p", g=8, s=32),
                        in1=qrt.rearrange("c (g p) -> c g p", g=8)
                            .unsqueeze(2).to_broadcast([3, 8, 32, 16]),
                        op=OP.subtract)

                # fold chunks into pairs: featp rows 0:4 = even chunk,
                # rows 4:8 = odd chunk, pair u at free 512u
                featp = fppool.tile([8, 4096], F16, tag="featp")
                h["featp"] = featp
                nc.gpsimd.dma_start(
                    out=featp[0:4, :].rearrange("c (u f) -> c u f", u=8),
                    in_=feat[0:4, :].rearrange("c (u f) -> c u f", u=8)
                        [:, :, 0:512])
                nc.gpsimd.dma_start(
                    out=featp[4:8, :].rearrange("c (u f) -> c u f", u=8),
                    in_=feat[0:4, :].rearrange("c (u f) -> c u f", u=8)
                        [:, :, 512:1024])

            def conv_stage(t):
                h = tiles[t]
                featp = h["featp"]
                # conv1: 8 paired matmuls -> y1 packed [128, 4096]
                y1 = yp.tile([128, 4096], F16, tag="y1")
                for u in range(8):
                    pm1 = pc1.tile([128, 512], F32, tag="pm1")
                    nc.tensor.matmul(
                        out=pm1[:], lhsT=w1b[:],
                        rhs=featp[:, u * 512:(u + 1) * 512],
                        start=True, stop=True)
                    # stats on the scalar engine (P1's DVE is saturated by
                    # the topk): sum rides the copy, sumsq via Square
                    s_ = t * 8 + u
                    nc.scalar.activation(
                        out=y1[:, u * 512:(u + 1) * 512], in_=pm1[:],
                        func=AF.Identity, accum_out=sm1[:, s_:s_ + 1])
                    sqs = wp.tile([128, 512], F16, tag="sqs")
                    nc.scalar.activation(
                        out=sqs[:], in_=pm1[:], func=AF.Square,
                        accum_out=sq1[:, s_:s_ + 1])
                nc.sync.dma_start(out=d["y1d"][t], in_=y1[:])
                h.clear()

            for t in range(NT):
                knn_stage(t)
                if t >= 1:
                    feat_stage(t - 1)
                if t >= 2:
                    conv_stage(t - 2)
            feat_stage(NT - 1)
            conv_stage(NT - 2)
            conv_stage(NT - 1)

        _bn_fold_raw(tc, 0, sm1, sq1, gb1, ab1, d["arin0"], d["arout0"], C1)

        # ---------------- Phase 2: apply BN1+relu, conv2 ----------------
        with tc.tile_pool(name="p2y", bufs=2) as yp, \
             tc.tile_pool(name="p2psum", bufs=6, space="PSUM") as cp:
            for t in range(NT):
                y1 = yp.tile([128, 4096], F16, tag="y1l")
                nc.sync.dma_start(out=y1[:], in_=d["y1d"][t])
                nc.scalar.activation(
                    out=y1[:], in_=y1[:], func=AF.Relu,
                    scale=ab1[:, 0:1], bias=ab1[:, 1:2])
                y2 = yp.tile([128, 4096], F16, tag="y2")
                for u in range(8):
                    pm = cp.tile([128, 512], F32, tag="pm2")
                    nc.tensor.matmul(
                        out=pm[:], lhsT=w2b[:],
                        rhs=y1[:, u * 512:(u + 1) * 512],
                        start=True, stop=True)
                    nc.vector.bn_stats(
                        out=st2[:, (t * 8 + u) * 6:(t * 8 + u + 1) * 6],
                        in_=pm[:])
                    # 5 copies on scalar, 3 on vector: P2 is scalar-bound
                    if u % 3 != 2:
                        nc.scalar.activation(
                            out=y2[:, u * 512:(u + 1) * 512], in_=pm[:],
                            func=AF.Identity)
                    else:
                        nc.vector.tensor_copy(
                            out=y2[:, u * 512:(u + 1) * 512], in_=pm[:])
                nc.sync.dma_start(out=d["y2d"][t], in_=y2[:])

        _bn_fold(tc, 1, st2, gb2, ab2, d["arin1"], d["arout1"], C2)

        # ---------------- Phase 3: apply BN2+relu, conv3 ----------------
        with tc.tile_pool(name="p3y", bufs=2) as yp, \
             tc.tile_pool(name="p3psum", bufs=6, space="PSUM") as cp:
            for t in range(NT):
                y2 = yp.tile([128, 4096], F16, tag="y2l")
                nc.sync.dma_start(out=y2[:], in_=d["y2d"][t])
                nc.scalar.activation(
                    out=y2[:], in_=y2[:], func=AF.Relu,
                    scale=ab2[:, 0:1], bias=ab2[:, 1:2])
                y3 = yp.tile([128, 8192], F16, tag="y3")
                for c in range(16):
                    bp_ = 64 * (c % 2)
                    pm = cp.tile([C3, 512], F32, tag="pm3")
                    nc.tensor.matmul(
                        out=pm[:], lhsT=w3d[bp_:bp_ + 64, :],
                        rhs=y2[bp_:bp_ + 64,
                               512 * (c // 2):512 * (c // 2) + 512],
                        start=True, stop=True)
                    nc.vector.bn_stats(
                        out=st3[:, (t * 16 + c) * 6:(t * 16 + c + 1) * 6],
                        in_=pm[:])
                    # 12 copies on scalar, 4 on vector: balances S vs the
                    # bn_stats-loaded DVE
                    if c % 4 != 3:
                        nc.scalar.activation(
                            out=y3[:, c * 512:(c + 1) * 512], in_=pm[:],
                            func=AF.Identity)
                    else:
                        nc.vector.tensor_copy(
                            out=y3[:, c * 512:(c + 1) * 512], in_=pm[:])
                nc.sync.dma_start(out=d["y3d"][t], in_=y3[:])

        _bn_fold(tc, 2, st3, gb3, ab3, d["arin2"], d["arout2"], C3)

        # ------------- Phase 4: scores, softmax, fusion, output -------------
        with tc.tile_pool(name="p4z", bufs=2) as zp, \
             tc.tile_pool(name="p4zw", bufs=2) as zw, \
             tc.tile_pool(name="p4work", bufs=2) as wp, \
             tc.tile_pool(name="p4psum", bufs=2, space="PSUM") as pp4, \
             tc.tile_pool(name="p4out", bufs=1) as op_:
            outsb = op_.tile([4, QPC], F32)
            for t in range(NT):
                z = zp.tile([128, 8192], F16, tag="z")
                nc.sync.dma_start(out=z[:], in_=d["y3d"][t])
                # per-channel affine a3*z + b3 (relu deferred past the max)
                nc.scalar.activation(
                    out=z[:], in_=z[:], func=AF.Identity,
                    scale=ab3[:, 0:1], bias=ab3[:, 1:2])
                # channel max: stream-transpose 32x32 blocks first, reduce
                # each block over free (split DVE/GPSIMD), then fold the 4
                # partition groups on small [*, 256] tiles via DMA shifts
                # (engines need same-start-partition operands)
                zT = zw.tile([128, 8192], F16, tag="zT")
                nc.vector.transpose(out=zT[:], in_=z[:])
                R = wp.tile([128, 256], F16, tag="R")
                nc.vector.tensor_reduce(
                    out=R[:],
                    in_=zT[:].rearrange("c (j e) -> c j e", e=32),
                    axis=AX.X, op=OP.max)
                Rs = wp.tile([64, 256], F16, tag="Rs")
                nc.scalar.dma_start(out=Rs[:], in_=R[64:128, :])
                R2 = wp.tile([64, 256], F16, tag="R2")
                nc.vector.tensor_tensor(out=R2[:], in0=R[0:64, :],
                                        in1=Rs[:], op=OP.max)
                R2s = wp.tile([32, 256], F16, tag="R2s")
                nc.scalar.dma_start(out=R2s[:], in_=R2[32:64, :])
                T32 = wp.tile([32, 256], F16, tag="T32")
                nc.vector.tensor_tensor(out=T32[:], in0=R2[0:32, :],
                                        in1=R2s[:], op=OP.max)
                # relu (commutes with the channel max)
                nc.vector.tensor_scalar_max(T32[:], T32[:], 0.0)
                # assemble raw scores per knn half in q-major layout:
                # scX[g, q*32 + sl*16 + sh] <- T32[16*sl+q, 16*(kn*8+g)+sh]
                # (slot s = 2*sh + sl)
                scA = wp.tile([8, 512], F16, tag="scA")
                scB = wp.tile([8, 512], F16, tag="scB")
                for kn, sct in ((0, scA), (1, scB)):
                    for g in range(8):
                        cbase = 16 * (kn * 8 + g)
                        ov = sct[g:g + 1, :].rearrange(
                            "c (q sl sh) -> c sl q sh", q=16, sl=2)
                        for sl in (0, 1):
                            eng = nc.sync if (g + sl) % 2 == 0 else nc.gpsimd
                            eng.dma_start(
                                out=ov[:, sl],
                                in_=T32[16 * sl:16 * sl + 16,
                                        cbase:cbase + 16])
                # per-query max over the 64 slots, subtract, exponentiate
                qmA = wp.tile([8, 16], F16, tag="qmA")
                qmB = wp.tile([8, 16], F16, tag="qmB")
                for sct, qm in ((scA, qmA), (scB, qmB)):
                    nc.vector.tensor_reduce(
                        out=qm[:],
                        in_=sct[:].rearrange("c (q z) -> c q z", z=32),
                        axis=AX.X, op=OP.max)
                nc.vector.tensor_tensor(out=qmA[:], in0=qmA[:], in1=qmB[:],
                                        op=OP.max)
                for sct in (scA, scB):
                    nc.vector.tensor_tensor(
                        out=sct[:].rearrange("c (q z) -> c q z", z=32),
                        in0=sct[:].rearrange("c (q z) -> c q z", z=32),
                        in1=qmA[:].unsqueeze(2).to_broadcast([8, 16, 32]),
                        op=OP.subtract)
                    nc.scalar.activation(out=sct[:], in_=sct[:], func=AF.Exp)
                # denominators over the 64 slots of each query
                qsA = wp.tile([8, 16], F32, tag="qsA")
                qsB = wp.tile([8, 16], F32, tag="qsB")
                for sct, qs in ((scA, qsA), (scB, qsB)):
                    nc.vector.tensor_reduce(
                        out=qs[:],
                        in_=sct[:].rearrange("c (q z) -> c q z", z=32),
                        axis=AX.X, op=OP.add)
                nc.vector.tensor_tensor(out=qsA[:], in0=qsA[:], in1=qsB[:],
                                        op=OP.add)
                nc.vector.reciprocal(out=qsA[:], in_=qsA[:])
                rec = wp.tile([8, 16], F16, tag="rec")
                nc.vector.tensor_copy(out=rec[:], in_=qsA[:])
                for sct in (scA, scB):
                    nc.vector.tensor_tensor(
                        out=sct[:].rearrange("c (q z) -> c q z", z=32),
                        in0=sct[:].rearrange("c (q z) -> c q z", z=32),
                        in1=rec[:].unsqueeze(2).to_broadcast([8, 16, 32]),
                        op=OP.mult)
                # replicate weight rows onto band partitions, multiply with
                # raw coords, segment-reduce over slots
                wr1 = wp.tile([128, 512], F16, tag="wr1")
                wr2 = wp.tile([128, 512], F16, tag="wr2")
                for sct, wr in ((scA, wr1), (scB, wr2)):
                    pw = pp4.tile([128, 512], F32, tag="pw")
                    nc.tensor.matmul(
                        out=pw[:], lhsT=selw[:], rhs=sct[:],
                        start=True, stop=True)
                    nc.scalar.activation(out=wr[:], in_=pw[:],
                                         func=AF.Identity)
                # coords are in o = s*16+q layout -> view them q-major to
                # line up with wr (q-major from the selector matmul)
                g1l = wp.tile([128, 512], F16, tag="g1l")
                g2l = wp.tile([128, 512], F16, tag="g2l")
                nc.sync.dma_start(out=g1l[:], in_=d["g1d"][t])
                nc.sync.dma_start(out=g2l[:], in_=d["g2d"][t])
                pr = wp.tile([128, 512], F16, tag="pr")
                gv1 = g1l[:].rearrange("c (sh sl q) -> c q sl sh",
                                       sh=16, sl=2)
                gv2 = g2l[:].rearrange("c (sh sl q) -> c q sl sh",
                                       sh=16, sl=2)
                wv1 = wr1[:].rearrange("c (q sl sh) -> c q sl sh",
                                       q=16, sl=2)
                wv2 = wr2[:].rearrange("c (q sl sh) -> c q sl sh",
                                       q=16, sl=2)
                pv = pr[:].rearrange("c (q sl sh) -> c q sl sh", q=16, sl=2)
                nc.vector.tensor_tensor(out=pv, in0=gv1, in1=wv1, op=OP.mult)
                nc.gpsimd.tensor_tensor(out=wv2, in0=gv2, in1=wv2,
                                        op=OP.mult)
                nc.vector.tensor_tensor(out=pr[:], in0=pr[:], in1=wr2[:],
                                        op=OP.add)
                fp_ = wp.tile([128, 16], F32, tag="fp")
                nc.vector.tensor_reduce(
                    out=fp_[:], in_=pr[:].rearrange("c (q z) -> c q z", z=32),
                    axis=AX.X, op=OP.add)
                for g in range(8):
                    nc.scalar.dma_start(
                        out=outsb[0:3,
                                  t * 128 + 16 * g: t * 128 + 16 * g + 16],
                        in_=fp_[16 * g: 16 * g + 3, :])
            nc.sync.dma_start(out=d["out"][:], in_=outsb[0:3, :])


def _bn_fold_raw(tc, li, sm, sq, gbe, ab, arin, arout, C):
    """Like _bn_fold but from raw per-slot (sum, sumsq) accumulators."""
    nc = tc.nc
    ntot = NTOT / (NCORES if getattr(nc, "_single_core_nocoll", False) else 1)
    with tc.tile_pool(name=f"bnr{li}", bufs=1) as bp:
        ss = bp.tile([128, 2], F32)
        nc.vector.tensor_reduce(out=ss[:, 0:1], in_=sm[:], axis=AX.X,
                                op=OP.add)
        nc.vector.tensor_reduce(out=ss[:, 1:2], in_=sq[:], axis=AX.X,
                                op=OP.add)
        sh = bp.tile([C, 2], F32)
        nc.sync.dma_start(out=sh[:], in_=ss[C:2 * C, :])
        sc = bp.tile([C, 2], F32)
        nc.vector.tensor_tensor(out=sc[:], in0=ss[0:C, :], in1=sh[:],
                                op=OP.add)
        nc.sync.dma_start(out=arin[:], in_=sc[:])
        if getattr(nc, "_single_core_nocoll", False):
            nc.sync.dma_start(out=arout[:], in_=arin[:])
        else:
            nc.gpsimd.collective_compute(
                "AllReduce", OP.add, replica_groups=[list(range(NCORES))],
                ins=[arin.opt()], outs=[arout.opt()])
        ar = bp.tile([C, 2], F32)
        nc.sync.dma_start(out=ar[:], in_=arout[:])
        mean = bp.tile([C, 1], F32)
        var = bp.tile([C, 1], F32)
        nc.vector.tensor_scalar_mul(mean[:], ar[:, 0:1], 1.0 / ntot)
        nc.vector.tensor_scalar_mul(var[:], ar[:, 1:2], 1.0 / ntot)
        mm = bp.tile([C, 1], F32)
        nc.vector.tensor_tensor(out=mm[:], in0=mean[:], in1=mean[:],
                                op=OP.mult)
        nc.vector.tensor_tensor(out=var[:], in0=var[:], in1=mm[:],
                                op=OP.subtract)
        nc.vector.tensor_scalar_add(var[:], var[:], BN_EPS)
        nc.scalar.activation(out=var[:], in_=var[:], func=AF.Sqrt)
        nc.vector.reciprocal(out=var[:], in_=var[:])  # rsqrt(var+eps)
        nc.vector.tensor_tensor(out=ab[0:C, 0:1], in0=var[:],
                                in1=gbe[:, 0:1], op=OP.mult)     # a
        nc.vector.tensor_tensor(out=mm[:], in0=ab[0:C, 0:1], in1=mean[:],
                                op=OP.mult)
        nc.vector.tensor_tensor(out=ab[0:C, 1:2], in0=gbe[:, 1:2], in1=mm[:],
                                op=OP.subtract)       # b = be - a*mean
        nc.vector.tensor_copy(out=ab[C:2 * C, :], in_=ab[0:C, :])


def _bn_fold(tc, li, st, gbe, ab, arin, arout, C):
    """bn_aggr per partition, convert to (sum, sumsq), fold dup halves for
    64-ch layers, AllReduce, then a = g*rsqrt(var+eps), b = be - a*mean."""
    nc = tc.nc
    n_loc = float(QPC * 64 * C // 128)  # pixels per partition slot
    ntot = NTOT / (NCORES if getattr(nc, "_single_core_nocoll", False) else 1)
    with tc.tile_pool(name=f"bn{li}", bufs=1) as bp:
        ag = bp.tile([128, 2], F32)
        nc.vector.bn_aggr(out=ag[:], in_=st[:])
        ss = bp.tile([128, 2], F32)
        m2 = bp.tile([128, 1], F32)
        nc.vector.tensor_tensor(out=m2[:], in0=ag[:, 0:1], in1=ag[:, 0:1],
                                op=OP.mult)
        nc.vector.tensor_tensor(out=ss[:, 1:2], in0=ag[:, 1:2], in1=m2[:],
                                op=OP.add)            # var + mean^2
        nc.vector.tensor_scalar_mul(ss[:, 1:2], ss[:, 1:2], n_loc)
        nc.vector.tensor_scalar_mul(ss[:, 0:1], ag[:, 0:1], n_loc)
        if C == 64:
            sh = bp.tile([64, 2], F32)
            nc.sync.dma_start(out=sh[:], in_=ss[64:128, :])
            sc = bp.tile([64, 2], F32)
            nc.vector.tensor_tensor(out=sc[:], in0=ss[0:64, :],
                                    in1=sh[:], op=OP.add)
        else:
            sc = ss
        nc.sync.dma_start(out=arin[:], in_=sc[:])
        if getattr(nc, "_single_core_nocoll", False):
            nc.sync.dma_start(out=arout[:], in_=arin[:])
        else:
            nc.gpsimd.collective_compute(
                "AllReduce", OP.add, replica_groups=[list(range(NCORES))],
                ins=[arin.opt()], outs=[arout.opt()])
        ar = bp.tile([C, 2], F32)
        nc.sync.dma_start(out=ar[:], in_=arout[:])
        mean = bp.tile([C, 1], F32)
        var = bp.tile([C, 1], F32)
        nc.vector.tensor_scalar_mul(mean[:], ar[:, 0:1], 1.0 / ntot)
        nc.vector.tensor_scalar_mul(var[:], ar[:, 1:2], 1.0 / ntot)
        mm = bp.tile([C, 1], F32)
        nc.vector.tensor_tensor(out=mm[:], in0=mean[:], in1=mean[:],
                                op=OP.mult)
        nc.vector.tensor_tensor(out=var[:], in0=var[:], in1=mm[:],
                                op=OP.subtract)
        nc.vector.tensor_scalar_add(var[:], var[:], BN_EPS)
        nc.scalar.activation(out=var[:], in_=var[:], func=AF.Sqrt)
        nc.vector.reciprocal(out=var[:], in_=var[:])  # rsqrt(var+eps)
        nc.vector.tensor_tensor(out=ab[0:C, 0:1], in0=var[:],
                                in1=gbe[:, 0:1], op=OP.mult)     # a
        nc.vector.tensor_tensor(out=mm[:], in0=ab[0:C, 0:1], in1=mean[:],
                                op=OP.mult)
        nc.vector.tensor_tensor(out=ab[0:C, 1:2], in0=gbe[:, 1:2], in1=mm[:],
                                op=OP.subtract)       # b = be - a*mean
        if C == 64:
            nc.vector.tensor_copy(out=ab[C:2 * C, :], in_=ab[0:C, :])


_PROGRAM = None
LAST_RESULT = None


def _get_program():
    global _PROGRAM
    if _PROGRAM is None:
        _PROGRAM = _build_program()
    return _PROGRAM


def _split16(x):
    hi = x.astype(np.float16).astype(np.float32)
    return hi, (x - hi).astype(np.float32)


def _prep_core_inputs(points1, points2, W1, W2, W3, gs, bes, b, h):
    p1 = points1[b]          # [3, N]
    p2 = points2[b]
    q = p1[:, h * QPC:(h + 1) * QPC]            # [3, QPC]

    qhi, qlo = _split16(q)
    qf = np.concatenate([2.0 * qhi, 2.0 * qhi, 2.0 * qlo,
                         np.ones((2, QPC), np.float32)], axis=0)

    def cand_tab(p):
        chi, clo = _split16(p)
        csq = (p * p).sum(axis=0)
        cshi, cslo = _split16(csq)
        return np.concatenate([chi, clo, chi, -cshi[None], -cslo[None]],
                              axis=0).astype(np.float16)   # [11, N]

    gtab = np.zeros((128, N), np.float32)
    gtab2 = np.zeros((128, N), np.float32)
    for g in range(8):
        gtab[16 * g + 0:16 * g + 3] = p1
        gtab2[16 * g + 0:16 * g + 3] = p2
    qraw = np.zeros((4, QPC), np.float16)
    qraw[0:3] = q.astype(np.float16)
    nqsqv = -(q * q).sum(axis=0).reshape(NT, 128).T.astype(np.float32)

    w1t = np.ascontiguousarray(W1.T).astype(np.float16)    # [4, 64]
    w2t = np.ascontiguousarray(W2.T).astype(np.float16)    # [64, 64]
    w3t = np.ascontiguousarray(W3.T).astype(np.float16)    # [64, 128]
    w1blk = np.zeros((8, 128), np.float16)
    w1blk[0:4, 0:64] = w1t
    w1blk[4:8, 64:128] = w1t
    w2blk = np.zeros((128, 128), np.float16)
    w2blk[0:64, 0:64] = w2t
    w2blk[64:128, 64:128] = w2t
    w3dup = np.concatenate([w3t, w3t], axis=0).astype(np.float16)

    selw = np.zeros((8, 128), np.float16)
    for g in range(8):
        for c3 in range(3):
            selw[g, 16 * g + c3] = 1.0

    return {
        "qf": qf.astype(np.float16),
        "t1": cand_tab(p1), "t2": cand_tab(p2),
        "nqsq": np.ascontiguousarray(nqsqv),
        "gt": gtab, "gt2": gtab2, "qr": qraw,
        "w1b": w1blk, "w2b": w2blk, "w3d": w3dup,
        "gb1": np.stack([gs[0], bes[0]], axis=1).astype(np.float32),
        "gb2": np.stack([gs[1], bes[1]], axis=1).astype(np.float32),
        "gb3": np.stack([gs[2], bes[2]], axis=1).astype(np.float32),
        "selw": selw,
    }


def kernel(points1, points2, k, t, W1, b1, g1, be1, W2, b2, g2, be2,
           W3, b3, g3, be3):
    # b1/b2/b3 cancel inside train-mode BatchNorm; t is unused by the net.
    assert int(np.asarray(k)) == KNN
    points1 = np.asarray(points1, np.float32)
    points2 = np.asarray(points2, np.float32)
    gs = [np.asarray(g1, np.float32), np.asarray(g2, np.float32),
          np.asarray(g3, np.float32)]
    bes = [np.asarray(be1, np.float32), np.asarray(be2, np.float32),
           np.asarray(be3, np.float32)]
    Ws = [np.asarray(W1, np.float32), np.asarray(W2, np.float32),
          np.asarray(W3, np.float32)]

    in_maps = []
    for c in range(NCORES):
        b, h = divmod(c, 2)
        in_maps.append(_prep_core_inputs(points1, points2, *Ws, gs, bes, b, h))

    nc = _get_program()
    bkr = run_bass_kernel_spmd(nc, in_maps, list(range(NCORES)))
    global LAST_RESULT
    LAST_RESULT = bkr
    res = bkr.results

    out = np.zeros((B, 3, N), np.float32)
    for c in range(NCORES):
        b, h = divmod(c, 2)
        out[b, :, h * QPC:(h + 1) * QPC] = res[c]["out"]
    return out


# revision 45
# speedup vs baseline: 1.0743x; 1.0067x over previous
"""PointsFusion Trainium2 kernel (fp16 fast path).

Pipeline per batch b (B=4, N=4096, k=32):
  knn1 = 32-NN of p1 in p1, knn2 = 32-NN of p1 in p2 (exact, DVE 8-max rounds
  on fp32 recentred scores; dist matmul uses fp16 split-channel inputs so the
  fp32 PSUM result is exact to ~1e-6)
  gather neighbor coords, features (resi, dist) -> conv(4->64)->BN->relu
  -> conv(64->64)->BN->relu -> conv(64->128)->BN->relu -> channel-max scores
  -> softmax over 64 neighbors -> weighted sum of neighbor coords.

Sharding: 8 cores = (batch b, half h of the 4096 query points). BatchNorm uses
global batch stats -> 3 tiny AllReduces of per-channel sum/sumsq.

Layouts (per 128-query tile):
  pixel space: 16 chunks of 512; chunk c = kn*8+g, pixel o = s*16 + q
  (g = query group, q = query-in-group, s = neighbor slot, kn = which knn).
  64-ch activations y1/y2 are packed [128, 4096]: pair u = chunks (2u, 2u+1),
  chunk 2u on partitions 0:64, 2u+1 on 64:128, both at free 512u; conv1/conv2
  use block-diagonal weights so one matmul computes a whole pair.
  y3 is [128, 8192], chunk c at free 512c. Channel-max via DVE stream
  transpose (32x32 blocks) + free-axis reduce; softmax without max-subtract
  (scores bounded, exp(x-4)).

Self-contained: hardcodes shapes; no sibling imports.
"""

import sys

import numpy as np

for _p in ("/opt/trn_rl_repo", "/opt/pypackages"):
    if _p not in sys.path:
        sys.path.append(_p)

import concourse.bass as bass  # noqa: E402  (imported for side effects/typing)
import concourse.mybir as mybir  # noqa: E402
import concourse.tile as tile  # noqa: E402
from concourse import bacc, bass_isa  # noqa: E402
from concourse.bass_utils import run_bass_kernel_spmd  # noqa: E402
from concourse.masks import make_identity  # noqa: E402

F32 = mybir.dt.float32
F16 = mybir.dt.float16
U16 = mybir.dt.uint16
I16 = mybir.dt.int16
AF = mybir.ActivationFunctionType
OP = mybir.AluOpType
AX = mybir.AxisListType

NCORES = 8
B = 4
N = 4096          # candidate points per batch
KNN = 32          # neighbors per knn
QPC = 2048        # query points per core
NT = 16           # query tiles of 128 per core
C1, C2, C3 = 64, 64, 128
NTOT = float(B * N * 2 * KNN)   # BN stat count (global)
BN_EPS = 1e-3
NEG = -1.0e30
EXP_SHIFT = -4.0


def _build_program(single=False):
    nc = bacc.Bacc(
        "TRN2", target_bir_lowering=False, debug=False,
        num_devices=1 if single else NCORES,
    )
    nc._single_core_nocoll = single

    ap = {}
    def din(name, shape, dt=F16):
        ap[name] = nc.dram_tensor(name, shape, dt, kind="ExternalInput").ap()
    din("qf", [11, QPC])
    din("t1", [11, N])
    din("t2", [11, N])
    din("nqsq", [128, NT], F32)
    din("gt", [128, N], F32)
    din("gt2", [128, N], F32)
    din("qr", [4, QPC])
    din("w1b", [8, 128])
    din("w2b", [128, 128])
    din("w3d", [128, 128])
    din("gb1", [C1, 2], F32)
    din("gb2", [C2, 2], F32)
    din("gb3", [C3, 2], F32)
    din("selw", [8, 128])

    ap["out"] = nc.dram_tensor("out", [3, QPC], F32, kind="ExternalOutput").ap()

    ap["y1d"] = nc.dram_tensor("y1d", [NT, 128, 4096], F16).ap()
    ap["y2d"] = nc.dram_tensor("y2d", [NT, 128, 4096], F16).ap()
    ap["y3d"] = nc.dram_tensor("y3d", [NT, 128, 8192], F16).ap()
    ap["g1d"] = nc.dram_tensor("g1d", [NT, 128, 512], F16).ap()
    ap["g2d"] = nc.dram_tensor("g2d", [NT, 128, 512], F16).ap()
    for i, c in ((0, C1), (1, C2), (2, C3)):
        ap[f"arin{i}"] = nc.dram_tensor(f"arin{i}", [c * 2], F32).ap()
        ap[f"arout{i}"] = nc.dram_tensor(f"arout{i}", [c * 2], F32).ap()

    with tile.TileContext(nc) as tc:
        _kernel_body(tc, ap)
    nc.compile()
    return nc


def _kernel_body(tc, d):
    nc = tc.nc
    from contextlib import ExitStack

    ctx = ExitStack()
    with ctx:
        cpool = ctx.enter_context(tc.tile_pool(name="consts", bufs=1))
        w1b = cpool.tile([8, 128], F16)
        w2b = cpool.tile([128, 128], F16)
        w3d = cpool.tile([128, 128], F16)
        gb1 = cpool.tile([C1, 2], F32)
        gb2 = cpool.tile([C2, 2], F32)
        gb3 = cpool.tile([C3, 2], F32)
        selw = cpool.tile([8, 128], F16)
        for nm, sb in [("w1b", w1b), ("w2b", w2b), ("w3d", w3d),
                       ("gb1", gb1), ("gb2", gb2), ("gb3", gb3),
                       ("selw", selw)]:
            nc.sync.dma_start(out=sb[:], in_=d[nm][:])

        spool = ctx.enter_context(tc.tile_pool(name="stats", bufs=1))
        sm1 = spool.tile([128, NT * 8], F32)
        sq1 = spool.tile([128, NT * 8], F32)
        st2 = spool.tile([128, NT * 8 * 6], F32)
        st3 = spool.tile([128, NT * 16 * 6], F32)
        ab1 = spool.tile([128, 2], F32)   # col0 = scale a, col1 = bias b
        ab2 = spool.tile([128, 2], F32)
        ab3 = spool.tile([C3, 2], F32)

        # ---------------- Phase 1: knn + gather + feat + conv1 ----------------
        # 3-stage software pipeline so the Vector queue never stalls:
        # iteration t issues knn(t) | feat(t-1) | conv1(t-2).
        with tc.tile_pool(name="p1c", bufs=1) as c1p, \
             tc.tile_pool(name="p1m", bufs=2) as mpool, \
             tc.tile_pool(name="p1pq", bufs=2, space="PSUM") as pq, \
             tc.tile_pool(name="p1pc", bufs=3, space="PSUM") as pc1, \
             tc.tile_pool(name="p1pt", bufs=2, space="PSUM") as pt, \
             tc.tile_pool(name="p1feat", bufs=1) as fpool, \
             tc.tile_pool(name="p1fp", bufs=2) as fppool, \
             tc.tile_pool(name="p1work", bufs=2) as wp, \
             tc.tile_pool(name="p1y", bufs=2) as yp:
            qf = c1p.tile([11, QPC], F16)
            t1 = c1p.tile([11, N], F16)
            t2 = c1p.tile([11, N], F16)
            nqsq = c1p.tile([128, NT], F32)
            gt = c1p.tile([128, N], F32)
            gt2 = c1p.tile([128, N], F32)
            qr = c1p.tile([4, QPC], F16)
            ident = c1p.tile([128, 128], F16)
            make_identity(nc, ident[:])
            for nm, sb in [("qf", qf), ("t1", t1), ("t2", t2),
                           ("nqsq", nqsq), ("gt", gt), ("gt2", gt2),
                           ("qr", qr)]:
                nc.sync.dma_start(out=sb[:], in_=d[nm][:])

            tiles = [{} for _ in range(NT)]

            def knn_stage(t):
                h = tiles[t]
                vals = wp.tile([128, 64], F32, tag="vals")
                idxu = wp.tile([128, 64], U16, tag="idxu")
                idxi = wp.tile([128, 64], I16, tag="idxi")
                h["vals"], h["idxu"], h["idxi"] = vals, idxu, idxi
                # msb = 2 q.c - |c|^2 - |q|^2 = -d^2 (max == nearest)
                msbs = []
                for kn, tab in ((0, t1), (1, t2)):
                    msb = mpool.tile([128, N], F32, tag="msb")
                    for ch in range(8):
                        pm = pq.tile([128, 512], F32, tag="pm")
                        nc.tensor.matmul(
                            out=pm[:],
                            lhsT=qf[:, t * 128:(t + 1) * 128],
                            rhs=tab[:, ch * 512:(ch + 1) * 512],
                            start=True, stop=True,
                        )
                        nc.scalar.activation(
                            out=msb[:, ch * 512:(ch + 1) * 512], in_=pm[:],
                            func=AF.Identity, bias=nqsq[:, t:t + 1])
                    msbs.append(msb)
                # top-32 rounds, the two knn streams interleaved op-by-op
                # so each stream's intra-round latency hides under the other
                def sl(kn, r):
                    o = kn * 32 + r * 8
                    return vals[:, o:o + 8], idxu[:, o:o + 8]
                for r in range(4):
                    for kn in (0, 1):
                        v8, _ = sl(kn, r)
                        nc.vector.max(out=v8, in_=msbs[kn][:])
                    for kn in (0, 1):
                        v8, i8 = sl(kn, r)
                        nc.vector.max_index(out=i8, in_max=v8,
                                            in_values=msbs[kn][:])
                    if r < 3:
                        for kn in (0, 1):
                            v8, _ = sl(kn, r)
                            nc.vector.match_replace(
                                out=msbs[kn][:], in_to_replace=v8,
                                in_values=msbs[kn][:], imm_value=NEG)
                nc.vector.tensor_copy(out=idxi[:], in_=idxu[:])

            def feat_stage(t):
                h = tiles[t]
                vals, idxi = h["vals"], h["idxi"]
                # gather neighbor coords; both tables carry xyz on band rows
                # 16g+{0..2}; convert to fp16 and spill for the fusion phase
                g1 = wp.tile([128, 512], F32, tag="g1")
                g2 = wp.tile([128, 512], F32, tag="g2")
                nc.gpsimd.ap_gather(
                    out_ap=g1[:], in_ap=gt[:], idxs_ap=idxi[:, 0:32],
                    channels=128, num_elems=N, d=1, num_idxs=512)
                nc.gpsimd.ap_gather(
                    out_ap=g2[:], in_ap=gt2[:], idxs_ap=idxi[:, 32:64],
                    channels=128, num_elems=N, d=1, num_idxs=512)
                g1h = wp.tile([128, 512], F16, tag="g1h")
                g2h = wp.tile([128, 512], F16, tag="g2h")
                nc.scalar.activation(out=g1h[:], in_=g1[:], func=AF.Identity)
                nc.scalar.activation(out=g2h[:], in_=g2[:], func=AF.Identity)
                nc.sync.dma_start(out=d["g1d"][t], in_=g1h[:])
                nc.sync.dma_start(out=d["g2d"][t], in_=g2h[:])

                # feat (flat): chunk c at free 512c; band rows -> coord rows
                feat = fpool.tile([4, 8192], F16, tag="feat")
                for g in range(8):
                    nc.scalar.dma_start(
                        out=feat[0:3, g * 512:(g + 1) * 512],
                        in_=g1h[16 * g: 16 * g + 3, :])
                    nc.scalar.dma_start(
                        out=feat[0:3, (8 + g) * 512:(9 + g) * 512],
                        in_=g2h[16 * g: 16 * g + 3, :])

                # dist = sqrt(relu(-vals)) into feat row 3 (pixel layout via
                # PE transpose then per-chunk strided DMAs)
                d2h = wp.tile([128, 64], F16, tag="d2h")
                nc.scalar.activation(out=d2h[:], in_=vals[:], func=AF.Relu,
                                     scale=-1.0)
                nc.scalar.activation(out=d2h[:], in_=d2h[:], func=AF.Sqrt)
                dtp = pt.tile([64, 128], F16, tag="dtp")
                nc.tensor.transpose(out=dtp[:], in_=d2h[:], identity=ident[:])
                d2t = wp.tile([64, 128], F16, tag="d2t")
                nc.scalar.activation(out=d2t[:], in_=dtp[:], func=AF.Identity)
                for kn in (0, 1):
                    for g in range(8):
                        c = kn * 8 + g
                        nc.gpsimd.dma_start(
                            out=feat[3:4, c * 512:(c + 1) * 512]
                                .rearrange("c (s p) -> c s p", s=32),
                            in_=d2t[kn * 32:(kn + 1) * 32,
                                    16 * g:16 * g + 16])

                # resi = nn - q (in place on coord rows; gpsimd -> off the
                # DVE critical path)
                qrt = qr[0:3, t * 128:(t + 1) * 128]
                for kn in (0, 1):
                    nc.gpsimd.tensor_tensor(
                        out=feat[0:3, kn * 4096:(kn + 1) * 4096]
                            .rearrange("c (g s p) -> c g s p", g=8, s=32),
                        in0=feat[0:3, kn * 4096:(kn + 1) * 4096]
                            .rearrange("c (g s p) -> c g s p", g=8, s=32),
                        in1=qrt.rearrange("c (g p) -> c g p", g=8)
                            .unsqueeze(2).to_broadcast([3, 8, 32, 16]),
                        op=OP.subtract)

                # fold chunks into pairs: featp rows 0:4 = even chunk,
                # rows 4:8 = odd chunk, pair u at free 512u
                featp = fppool.tile([8, 4096], F16, tag="featp")
                h["featp"] = featp
                nc.gpsimd.dma_start(
                    out=featp[0:4, :].rearrange("c (u f) -> c u f", u=8),
                    in_=feat[0:4, :].rearrange("c (u f) -> c u f", u=8)
                        [:, :, 0:512])
                nc.gpsimd.dma_start(
                    out=featp[4:8, :].rearrange("c (u f) -> c u f", u=8),
                    in_=feat[0:4, :].rearrange("c (u f) -> c u f", u=8)
                        [:, :, 512:1024])

            def conv_stage(t):
                h = tiles[t]
                featp = h["featp"]
                # conv1: 8 paired matmuls -> y1 packed [128, 4096]
                y1 = yp.tile([128, 4096], F16, tag="y1")
                for u in range(8):
                    pm1 = pc1.tile([128, 512], F32, tag="pm1")
                    nc.tensor.matmul(
                        out=pm1[:], lhsT=w1b[:],
                        rhs=featp[:, u * 512:(u + 1) * 512],
                        start=True, stop=True)
                    # stats on the scalar engine (P1's DVE is saturated by
                    # the topk): sum rides the copy, sumsq via Square
                    s_ = t * 8 + u
                    nc.scalar.activation(
                        out=y1[:, u * 512:(u + 1) * 512], in_=pm1[:],
                        func=AF.Identity, accum_out=sm1[:, s_:s_ + 1])
                    sqs = wp.tile([128, 512], F16, tag="sqs")
                    nc.scalar.activation(
                        out=sqs[:], in_=pm1[:], func=AF.Square,
                        accum_out=sq1[:, s_:s_ + 1])
                nc.sync.dma_start(out=d["y1d"][t], in_=y1[:])
                h.clear()

            for t in range(NT):
                knn_stage(t)
                if t >= 1:
                    feat_stage(t - 1)
                if t >= 2:
                    conv_stage(t - 2)
            feat_stage(NT - 1)
            conv_stage(NT - 2)
            conv_stage(NT - 1)

        _bn_fold_raw(tc, 0, sm1, sq1, gb1, ab1, d["arin0"], d["arout0"], C1)

        # ---------------- Phase 2: apply BN1+relu, conv2 ----------------
        with tc.tile_pool(name="p2y", bufs=2) as yp, \
             tc.tile_pool(name="p2psum", bufs=6, space="PSUM") as cp:
            for t in range(NT):
                y1 = yp.tile([128, 4096], F16, tag="y1l")
                nc.sync.dma_start(out=y1[:], in_=d["y1d"][t])
                nc.scalar.activation(
                    out=y1[:], in_=y1[:], func=AF.Relu,
                    scale=ab1[:, 0:1], bias=ab1[:, 1:2])
                y2 = yp.tile([128, 4096], F16, tag="y2")
                for u in range(8):
                    pm = cp.tile([128, 512], F32, tag="pm2")
                    nc.tensor.matmul(
                        out=pm[:], lhsT=w2b[:],
                        rhs=y1[:, u * 512:(u + 1) * 512],
                        start=True, stop=True)
                    nc.vector.bn_stats(
                        out=st2[:, (t * 8 + u) * 6:(t * 8 + u + 1) * 6],
                        in_=pm[:])
                    # 5 copies on scalar, 3 on vector: P2 is scalar-bound
                    if u % 3 != 2:
                        nc.scalar.activation(
                            out=y2[:, u * 512:(u + 1) * 512], in_=pm[:],
                            func=AF.Identity)
                    else:
                        nc.vector.tensor_copy(
                            out=y2[:, u * 512:(u + 1) * 512], in_=pm[:])
                nc.sync.dma_start(out=d["y2d"][t], in_=y2[:])

        _bn_fold(tc, 1, st2, gb2, ab2, d["arin1"], d["arout1"], C2)

        # ---------------- Phase 3: apply BN2+relu, conv3 ----------------
        with tc.tile_pool(name="p3y", bufs=2) as yp, \
             tc.tile_pool(name="p3psum", bufs=6, space="PSUM") as cp:
            for t in range(NT):
                y2 = yp.tile([128, 4096], F16, tag="y2l")
                nc.sync.dma_start(out=y2[:], in_=d["y2d"][t])
                nc.scalar.activation(
                    out=y2[:], in_=y2[:], func=AF.Relu,
                    scale=ab2[:, 0:1], bias=ab2[:, 1:2])
                y3 = yp.tile([128, 8192], F16, tag="y3")
                for c in range(16):
                    bp_ = 64 * (c % 2)
                    pm = cp.tile([C3, 512], F32, tag="pm3")
                    nc.tensor.matmul(
                        out=pm[:], lhsT=w3d[bp_:bp_ + 64, :],
                        rhs=y2[bp_:bp_ + 64,
                               512 * (c // 2):512 * (c // 2) + 512],
                        start=True, stop=True)
                    nc.vector.bn_stats(
                        out=st3[:, (t * 16 + c) * 6:(t * 16 + c + 1) * 6],
                        in_=pm[:])
                    # 12 copies on scalar, 4 on vector: balances S vs the
                    # bn_stats-loaded DVE
                    if c % 4 != 3:
                        nc.scalar.activation(
                            out=y3[:, c * 512:(c + 1) * 512], in_=pm[:],
                            func=AF.Identity)
                    else:
                        nc.vector.tensor_copy(
                            out=y3[:, c * 512:(c + 1) * 512], in_=pm[:])
                nc.sync.dma_start(out=d["y3d"][t], in_=y3[:])

        _bn_fold(tc, 2, st3, gb3, ab3, d["arin2"], d["arout2"], C3)

        # ------------- Phase 4: scores, softmax, fusion, output -------------
        with tc.tile_pool(name="p4z", bufs=2) as zp, \
             tc.tile_pool(name="p4zw", bufs=2) as zw, \
             tc.tile_pool(name="p4work", bufs=2) as wp, \
             tc.tile_pool(name="p4psum", bufs=2, space="PSUM") as pp4, \
             tc.tile_pool(name="p4out", bufs=1) as op_:
            outsb = op_.tile([4, QPC], F32)
            for t in range(NT):
                z = zp.tile([128, 8192], F16, tag="z")
                nc.sync.dma_start(out=z[:], in_=d["y3d"][t])
                # per-channel affine a3*z + b3 (relu deferred past the max)
                nc.scalar.activation(
                    out=z[:], in_=z[:], func=AF.Identity,
                    scale=ab3[:, 0:1], bias=ab3[:, 1:2])
                # channel max: stream-transpose 32x32 blocks first, reduce
                # each block over free (split DVE/GPSIMD), then fold the 4
                # partition groups on small [*, 256] tiles via DMA shifts
                # (engines need same-start-partition operands)
                zT = zw.tile([128, 8192], F16, tag="zT")
                nc.vector.transpose(out=zT[:], in_=z[:])
                R = wp.tile([128, 256], F16, tag="R")
                nc.vector.tensor_reduce(
                    out=R[:],
                    in_=zT[:].rearrange("c (j e) -> c j e", e=32),
                    axis=AX.X, op=OP.max)
                Rs = wp.tile([64, 256], F16, tag="Rs")
                nc.scalar.dma_start(out=Rs[:], in_=R[64:128, :])
                R2 = wp.tile([64, 256], F16, tag="R2")
                nc.vector.tensor_tensor(out=R2[:], in0=R[0:64, :],
                                        in1=Rs[:], op=OP.max)
                R2s = wp.tile([32, 256], F16, tag="R2s")
                nc.scalar.dma_start(out=R2s[:], in_=R2[32:64, :])
                T32 = wp.tile([32, 256], F16, tag="T32")
                nc.vector.tensor_tensor(out=T32[:], in0=R2[0:32, :],
                                        in1=R2s[:], op=OP.max)
                # relu (commutes with the channel max)
                nc.vector.tensor_scalar_max(T32[:], T32[:], 0.0)
                # assemble raw scores per knn half in q-major layout:
                # scX[g, q*32 + sl*16 + sh] <- T32[16*sl+q, 16*(kn*8+g)+sh]
                # (slot s = 2*sh + sl)
                scA = wp.tile([8, 512], F16, tag="scA")
                scB = wp.tile([8, 512], F16, tag="scB")
                for kn, sct in ((0, scA), (1, scB)):
                    for g in range(8):
                        cbase = 16 * (kn * 8 + g)
                        ov = sct[g:g + 1, :].rearrange(
                            "c (q sl sh) -> c sl q sh", q=16, sl=2)
                        for sl in (0, 1):
                            eng = nc.sync if (g + sl) % 2 == 0 else nc.gpsimd
                            eng.dma_start(
                                out=ov[:, sl],
                                in_=T32[16 * sl:16 * sl + 16,
                                        cbase:cbase + 16])
                # per-query max over the 64 slots, subtract, exponentiate
                qmA = wp.tile([8, 16], F16, tag="qmA")
                qmB = wp.tile([8, 16], F16, tag="qmB")
                for sct, qm in ((scA, qmA), (scB, qmB)):
                    nc.vector.tensor_reduce(
                        out=qm[:],
                        in_=sct[:].rearrange("c (q z) -> c q z", z=32),
                        axis=AX.X, op=OP.max)
                nc.vector.tensor_tensor(out=qmA[:], in0=qmA[:], in1=qmB[:],
                                        op=OP.max)
                for sct in (scA, scB):
                    nc.gpsimd.tensor_tensor(
                        out=sct[:].rearrange("c (q z) -> c q z", z=32),
                        in0=sct[:].rearrange("c (q z) -> c q z", z=32),
                        in1=qmA[:].unsqueeze(2).to_broadcast([8, 16, 32]),
                        op=OP.subtract)
                    nc.scalar.activation(out=sct[:], in_=sct[:], func=AF.Exp)
                # denominators over the 64 slots of each query
                qsA = wp.tile([8, 16], F32, tag="qsA")
                qsB = wp.tile([8, 16], F32, tag="qsB")
                for sct, qs in ((scA, qsA), (scB, qsB)):
                    nc.vector.tensor_reduce(
                        out=qs[:],
                        in_=sct[:].rearrange("c (q z) -> c q z", z=32),
                        axis=AX.X, op=OP.add)
                nc.vector.tensor_tensor(out=qsA[:], in0=qsA[:], in1=qsB[:],
                                        op=OP.add)
                nc.vector.reciprocal(out=qsA[:], in_=qsA[:])
                rec = wp.tile([8, 16], F16, tag="rec")
                nc.scalar.activation(out=rec[:], in_=qsA[:], func=AF.Identity)
                for sct in (scA, scB):
                    nc.gpsimd.tensor_tensor(
                        out=sct[:].rearrange("c (q z) -> c q z", z=32),
                        in0=sct[:].rearrange("c (q z) -> c q z", z=32),
                        in1=rec[:].unsqueeze(2).to_broadcast([8, 16, 32]),
                        op=OP.mult)
                # replicate weight rows onto band partitions, multiply with
                # raw coords, segment-reduce over slots
                wr1 = wp.tile([128, 512], F16, tag="wr1")
                wr2 = wp.tile([128, 512], F16, tag="wr2")
                for sct, wr in ((scA, wr1), (scB, wr2)):
                    pw = pp4.tile([128, 512], F32, tag="pw")
                    nc.tensor.matmul(
                        out=pw[:], lhsT=selw[:], rhs=sct[:],
                        start=True, stop=True)
                    nc.scalar.activation(out=wr[:], in_=pw[:],
                                         func=AF.Identity)
                # coords are in o = s*16+q layout -> view them q-major to
                # line up with wr (q-major from the selector matmul)
                g1l = wp.tile([128, 512], F16, tag="g1l")
                g2l = wp.tile([128, 512], F16, tag="g2l")
                nc.sync.dma_start(out=g1l[:], in_=d["g1d"][t])
                nc.sync.dma_start(out=g2l[:], in_=d["g2d"][t])
                pr = wp.tile([128, 512], F16, tag="pr")
                gv1 = g1l[:].rearrange("c (sh sl q) -> c q sl sh",
                                       sh=16, sl=2)
                gv2 = g2l[:].rearrange("c (sh sl q) -> c q sl sh",
                                       sh=16, sl=2)
                wv1 = wr1[:].rearrange("c (q sl sh) -> c q sl sh",
                                       q=16, sl=2)
                wv2 = wr2[:].rearrange("c (q sl sh) -> c q sl sh",
                                       q=16, sl=2)
                pv = pr[:].rearrange("c (q sl sh) -> c q sl sh", q=16, sl=2)
                nc.vector.tensor_tensor(out=pv, in0=gv1, in1=wv1, op=OP.mult)
                nc.gpsimd.tensor_tensor(out=wv2, in0=gv2, in1=wv2,
                                        op=OP.mult)
                nc.vector.tensor_tensor(out=pr[:], in0=pr[:], in1=wr2[:],
                                        op=OP.add)
                fp_ = wp.tile([128, 16], F32, tag="fp")
                nc.vector.tensor_reduce(
                    out=fp_[:], in_=pr[:].rearrange("c (q z) -> c q z", z=32),
                    axis=AX.X, op=OP.add)
                for g in range(8):
                    nc.scalar.dma_start(
                        out=outsb[0:3,
                                  t * 128 + 16 * g: t * 128 + 16 * g + 16],
                        in_=fp_[16 * g: 16 * g + 3, :])
            nc.sync.dma_start(out=d["out"][:], in_=outsb[0:3, :])


def _bn_fold_raw(tc, li, sm, sq, gbe, ab, arin, arout, C):
    """Like _bn_fold but from raw per-slot (sum, sumsq) accumulators."""
    nc = tc.nc
    ntot = NTOT / (NCORES if getattr(nc, "_single_core_nocoll", False) else 1)
    with tc.tile_pool(name=f"bnr{li}", bufs=1) as bp:
        ss = bp.tile([128, 2], F32)
        nc.vector.tensor_reduce(out=ss[:, 0:1], in_=sm[:], axis=AX.X,
                                op=OP.add)
        nc.vector.tensor_reduce(out=ss[:, 1:2], in_=sq[:], axis=AX.X,
                                op=OP.add)
        sh = bp.tile([C, 2], F32)
        nc.sync.dma_start(out=sh[:], in_=ss[C:2 * C, :])
        sc = bp.tile([C, 2], F32)
        nc.vector.tensor_tensor(out=sc[:], in0=ss[0:C, :], in1=sh[:],
                                op=OP.add)
        nc.sync.dma_start(out=arin[:], in_=sc[:])
        if getattr(nc, "_single_core_nocoll", False):
            nc.sync.dma_start(out=arout[:], in_=arin[:])
        else:
            nc.gpsimd.collective_compute(
                "AllReduce", OP.add, replica_groups=[list(range(NCORES))],
                ins=[arin.opt()], outs=[arout.opt()])
        ar = bp.tile([C, 2], F32)
        nc.sync.dma_start(out=ar[:], in_=arout[:])
        mean = bp.tile([C, 1], F32)
        var = bp.tile([C, 1], F32)
        nc.vector.tensor_scalar_mul(mean[:], ar[:, 0:1], 1.0 / ntot)
        nc.vector.tensor_scalar_mul(var[:], ar[:, 1:2], 1.0 / ntot)
        mm = bp.tile([C, 1], F32)
        nc.vector.tensor_tensor(out=mm[:], in0=mean[:], in1=mean[:],
                                op=OP.mult)
        nc.vector.tensor_tensor(out=var[:], in0=var[:], in1=mm[:],
                                op=OP.subtract)
        nc.vector.tensor_scalar_add(var[:], var[:], BN_EPS)
        nc.scalar.activation(out=var[:], in_=var[:], func=AF.Sqrt)
        nc.vector.reciprocal(out=var[:], in_=var[:])  # rsqrt(var+eps)
        nc.vector.tensor_tensor(out=ab[0:C, 0:1], in0=var[:],
                                in1=gbe[:, 0:1], op=OP.mult)     # a
        nc.vector.tensor_tensor(out=mm[:], in0=ab[0:C, 0:1], in1=mean[:],
                                op=OP.mult)
        nc.vector.tensor_tensor(out=ab[0:C, 1:2], in0=gbe[:, 1:2], in1=mm[:],
                                op=OP.subtract)       # b = be - a*mean
        nc.vector.tensor_copy(out=ab[C:2 * C, :], in_=ab[0:C, :])


def _bn_fold(tc, li, st, gbe, ab, arin, arout, C):
    """bn_aggr per partition, convert to (sum, sumsq), fold dup halves for
    64-ch layers, AllReduce, then a = g*rsqrt(var+eps), b = be - a*mean."""
    nc = tc.nc
    n_loc = float(QPC * 64 * C // 128)  # pixels per partition slot
    ntot = NTOT / (NCORES if getattr(nc, "_single_core_nocoll", False) else 1)
    with tc.tile_pool(name=f"bn{li}", bufs=1) as bp:
        ag = bp.tile([128, 2], F32)
        nc.vector.bn_aggr(out=ag[:], in_=st[:])
        ss = bp.tile([128, 2], F32)
        m2 = bp.tile([128, 1], F32)
        nc.vector.tensor_tensor(out=m2[:], in0=ag[:, 0:1], in1=ag[:, 0:1],
                                op=OP.mult)
        nc.vector.tensor_tensor(out=ss[:, 1:2], in0=ag[:, 1:2], in1=m2[:],
                                op=OP.add)            # var + mean^2
        nc.vector.tensor_scalar_mul(ss[:, 1:2], ss[:, 1:2], n_loc)
        nc.vector.tensor_scalar_mul(ss[:, 0:1], ag[:, 0:1], n_loc)
        if C == 64:
            sh = bp.tile([64, 2], F32)
            nc.sync.dma_start(out=sh[:], in_=ss[64:128, :])
            sc = bp.tile([64, 2], F32)
            nc.vector.tensor_tensor(out=sc[:], in0=ss[0:64, :],
                                    in1=sh[:], op=OP.add)
        else:
            sc = ss
        nc.sync.dma_start(out=arin[:], in_=sc[:])
        if getattr(nc, "_single_core_nocoll", False):
            nc.sync.dma_start(out=arout[:], in_=arin[:])
        else:
            nc.gpsimd.collective_compute(
                "AllReduce", OP.add, replica_groups=[list(range(NCORES))],
                ins=[arin.opt()], outs=[arout.opt()])
        ar = bp.tile([C, 2], F32)
        nc.sync.dma_start(out=ar[:], in_=arout[:])
        mean = bp.tile([C, 1], F32)
        var = bp.tile([C, 1], F32)
        nc.vector.tensor_scalar_mul(mean[:], ar[:, 0:1], 1.0 / ntot)
        nc.vector.tensor_scalar_mul(var[:], ar[:, 1:2], 1.0 / ntot)
        mm = bp.tile([C, 1], F32)
        nc.vector.tensor_tensor(out=mm[:], in0=mean[:], in1=mean[:],
                                op=OP.mult)
        nc.vector.tensor_tensor(out=var[:], in0=var[:], in1=mm[:],
                                op=OP.subtract)
        nc.vector.tensor_scalar_add(var[:], var[:], BN_EPS)
        nc.scalar.activation(out=var[:], in_=var[:], func=AF.Sqrt)
        nc.vector.reciprocal(out=var[:], in_=var[:])  # rsqrt(var+eps)
        nc.vector.tensor_tensor(out=ab[0:C, 0:1], in0=var[:],
                                in1=gbe[:, 0:1], op=OP.mult)     # a
        nc.vector.tensor_tensor(out=mm[:], in0=ab[0:C, 0:1], in1=mean[:],
                                op=OP.mult)
        nc.vector.tensor_tensor(out=ab[0:C, 1:2], in0=gbe[:, 1:2], in1=mm[:],
                                op=OP.subtract)       # b = be - a*mean
        if C == 64:
            nc.vector.tensor_copy(out=ab[C:2 * C, :], in_=ab[0:C, :])


_PROGRAM = None
LAST_RESULT = None


def _get_program():
    global _PROGRAM
    if _PROGRAM is None:
        _PROGRAM = _build_program()
    return _PROGRAM


def _split16(x):
    hi = x.astype(np.float16).astype(np.float32)
    return hi, (x - hi).astype(np.float32)


def _prep_core_inputs(points1, points2, W1, W2, W3, gs, bes, b, h):
    p1 = points1[b]          # [3, N]
    p2 = points2[b]
    q = p1[:, h * QPC:(h + 1) * QPC]            # [3, QPC]

    qhi, qlo = _split16(q)
    qf = np.concatenate([2.0 * qhi, 2.0 * qhi, 2.0 * qlo,
                         np.ones((2, QPC), np.float32)], axis=0)

    def cand_tab(p):
        chi, clo = _split16(p)
        csq = (p * p).sum(axis=0)
        cshi, cslo = _split16(csq)
        return np.concatenate([chi, clo, chi, -cshi[None], -cslo[None]],
                              axis=0).astype(np.float16)   # [11, N]

    gtab = np.zeros((128, N), np.float32)
    gtab2 = np.zeros((128, N), np.float32)
    for g in range(8):
        gtab[16 * g + 0:16 * g + 3] = p1
        gtab2[16 * g + 0:16 * g + 3] = p2
    qraw = np.zeros((4, QPC), np.float16)
    qraw[0:3] = q.astype(np.float16)
    nqsqv = -(q * q).sum(axis=0).reshape(NT, 128).T.astype(np.float32)

    w1t = np.ascontiguousarray(W1.T).astype(np.float16)    # [4, 64]
    w2t = np.ascontiguousarray(W2.T).astype(np.float16)    # [64, 64]
    w3t = np.ascontiguousarray(W3.T).astype(np.float16)    # [64, 128]
    w1blk = np.zeros((8, 128), np.float16)
    w1blk[0:4, 0:64] = w1t
    w1blk[4:8, 64:128] = w1t
    w2blk = np.zeros((128, 128), np.float16)
    w2blk[0:64, 0:64] = w2t
    w2blk[64:128, 64:128] = w2t
    w3dup = np.concatenate([w3t, w3t], axis=0).astype(np.float16)

    selw = np.zeros((8, 128), np.float16)
    for g in range(8):
        for c3 in range(3):
            selw[g, 16 * g + c3] = 1.0

    return {
        "qf": qf.astype(np.float16),
        "t1": cand_tab(p1), "t2": cand_tab(p2),
        "nqsq": np.ascontiguousarray(nqsqv),
        "gt": gtab, "gt2": gtab2, "qr": qraw,
        "w1b": w1blk, "w2b": w2blk, "w3d": w3dup,
        "gb1": np.stack([gs[0], bes[0]], axis=1).astype(np.float32),
        "gb2": np.stack([gs[1], bes[1]], axis=1).astype(np.float32),
        "gb3": np.stack([gs[2], bes[2]], axis=1).astype(np.float32),
        "selw": selw,
    }


def kernel(points1, points2, k, t, W1, b1, g1, be1, W2, b2, g2, be2,
           W3, b3, g3, be3):
    # b1/b2/b3 cancel inside train-mode BatchNorm; t is unused by the net.
    assert int(np.asarray(k)) == KNN
    points1 = np.asarray(points1, np.float32)
    points2 = np.asarray(points2, np.float32)
    gs = [np.asarray(g1, np.float32), np.asarray(g2, np.float32),
          np.asarray(g3, np.float32)]
    bes = [np.asarray(be1, np.float32), np.asarray(be2, np.float32),
           np.asarray(be3, np.float32)]
    Ws = [np.asarray(W1, np.float32), np.asarray(W2, np.float32),
          np.asarray(W3, np.float32)]

    in_maps = []
    for c in range(NCORES):
        b, h = divmod(c, 2)
        in_maps.append(_prep_core_inputs(points1, points2, *Ws, gs, bes, b, h))

    nc = _get_program()
    bkr = run_bass_kernel_spmd(nc, in_maps, list(range(NCORES)))
    global LAST_RESULT
    LAST_RESULT = bkr
    res = bkr.results

    out = np.zeros((B, 3, N), np.float32)
    for c in range(NCORES):
        b, h = divmod(c, 2)
        out[b, :, h * QPC:(h + 1) * QPC] = res[c]["out"]
    return out
